# revision 21
# baseline (speedup 1.0000x reference)
"""Trainium2 Bass kernel for the spiking CapsNet forward pass (nn_CapsNet).

Strategy (8 NeuronCores):
  Phase A (batch-parallel, 4 images/core):
    conv1 once (input is constant over the 5 timesteps), conv-layer membrane
    dynamics for all 5 steps upfront, then the expensive prim conv batched
    over all 5 timesteps in one weight-stationary PE pass.
  AllToAll: re-shard prim spikes from batch-split to route-split.
  Phase B (route-parallel, 144 routes/core, full batch):
    u_hat on PE per route (contraction over i=8), digit-caps membrane /
    trace / routing chain on DVE+GPSIMD with a small AllGather of the
    per-core s_j partials each step.
Host side: input re-layout (im2col, weight transposes) and the final
  classes = sqrt(sum_o (out_mem/T)^2) reduction.
"""

import numpy as np

N_CORES = 8
T = 5
B = 32
BL = B // N_CORES          # local batch (4)
R = 1152
RL = R // N_CORES          # local routes (144)
CO = 160                   # (o,c) pairs, ordered co = o*10 + c
DECAY = np.float32(0.2)
THRESH = np.float32(0.5)
DECAY_TR = np.float32(np.exp(np.float32(-1.0 / 1.5)))
ALPHA = np.float32(np.float32(0.0008) / np.float32(32.0))

_CACHE = {}


def _build_program(reps=1, stage="full", solo=False):
    import concourse.bass as bass
    import concourse.mybir as mybir
    import concourse.tile as tile
    from concourse import bacc

    Alu = mybir.AluOpType
    Act = mybir.ActivationFunctionType
    f32 = mybir.dt.float32
    bf16 = mybir.dt.bfloat16

    nc = bacc.Bacc("TRN2", target_bir_lowering=False, debug=False,
                   num_devices=N_CORES)

    # ---- external I/O (per-core values supplied by host) ----
    im2_d = nc.dram_tensor("im2", [81, BL * 400], f32, kind="ExternalInput")
    convw_d = nc.dram_tensor("convw", [81, 256], f32, kind="ExternalInput")
    convb_d = nc.dram_tensor("convb", [128, 2], f32, kind="ExternalInput")
    primw_d = nc.dram_tensor("primw", [81, 128, 512], bf16,
                             kind="ExternalInput")
    primb_d = nc.dram_tensor("primb", [128, 2], f32, kind="ExternalInput")
    w2g_d = nc.dram_tensor("w2g", [9, 128, 16 * CO], bf16,
                           kind="ExternalInput")
    biav_d = nc.dram_tensor("biav", [128, 2], f32, kind="ExternalInput")
    sela_d = nc.dram_tensor("sela", [128, 160], bf16, kind="ExternalInput")
    selt_d = nc.dram_tensor("selt", [32, 160], bf16, kind="ExternalInput")
    repl_d = nc.dram_tensor("repl", [32, 128], bf16, kind="ExternalInput")
    fold_d = nc.dram_tensor("fold", [128, 32], f32, kind="ExternalInput")
    outm_d = nc.dram_tensor("outm", [160, 32], f32, kind="ExternalOutput")
    dbg_d = nc.dram_tensor("dbg", [128, 512], f32, kind="ExternalOutput")

    # ---- internal DRAM ----
    # a2a: per-dest-rank blocks [dest 8][b_l 4][t 5][flat 1152]
    a2a_in = nc.dram_tensor("a2a_in", [N_CORES * BL * T * 1152], bf16)
    a2a_out = nc.dram_tensor("a2a_out", [N_CORES * BL * T * 1152], bf16)
    # per-step s_j exchange (AllReduce payload: 4096 main + 1024 tail)
    SJP = 5120
    sj_in = [nc.dram_tensor(f"sj_in{t}", [SJP], f32) for t in range(T - 1)]
    sj_out = [nc.dram_tensor(f"sj_out{t}", [SJP], f32,
                             addr_space="Shared") for t in range(T - 1)]

    def A(t, p0, pc, dims, foff=0):
        """Raw AP on tile/tensor t: partitions [p0, p0+pc), free dims list
        [(step, count), ...] in elements, plus extra free offset."""
        b = t if isinstance(t, bass.AP) else t[:]
        pitch = b.ap[0][0]
        return bass.AP(b.tensor, b.offset + p0 * pitch + foff,
                       [[pitch, pc]] + [list(d) for d in dims])

    def D(h, dims, off=0):
        """Raw AP on a DRAM tensor handle (flat element space)."""
        b = h[:]
        return bass.AP(b.tensor, off, [list(d) for d in dims])

    rg = [list(range(N_CORES))]

    with tile.TileContext(nc) as tc:
        from contextlib import ExitStack
        for _rep in range(reps):
            _run_once(nc, tc, bass, mybir, locals(), stage, solo)

    nc.compile()
    return nc


def _run_once(nc, tc, bass, mybir, env, stage="full", solo=False):
    import numpy as np
    from contextlib import ExitStack
    Alu = mybir.AluOpType
    f32 = mybir.dt.float32
    bf16 = mybir.dt.bfloat16
    im2_d = env["im2_d"]; convw_d = env["convw_d"]; convb_d = env["convb_d"]
    primw_d = env["primw_d"]; primb_d = env["primb_d"]
    biav_d = env["biav_d"]; sela_d = env["sela_d"]
    repl_d = env["repl_d"]; fold_d = env["fold_d"]
    selt_d = env["selt_d"]; outm_d = env["outm_d"]; dbg_d = env["dbg_d"]
    a2a_in = env["a2a_in"]; a2a_out = env["a2a_out"]
    w2g_d = env["w2g_d"]
    sj_in = env["sj_in"]; sj_out = env["sj_out"]; SJP = env["SJP"]
    A = env["A"]; D = env["D"]; rg = env["rg"]

    if True:
        with ExitStack() as stk:
            # ---------------- persistent pools ----------------
            persist = stk.enter_context(tc.tile_pool(name="persist", bufs=1))
            dram = stk.enter_context(tc.tile_pool(name="dram", bufs=1,
                                                  space="DRAM"))

            # =========== Phase A: conv stage (batch-sharded) ===========
            with ExitStack() as cstk:
                cpool = cstk.enter_context(tc.tile_pool(name="conv", bufs=1))
                wpool = cstk.enter_context(tc.tile_pool(name="wpos", bufs=4))
                cps = cstk.enter_context(tc.tile_pool(name="cpsum", bufs=4,
                                                      space="PSUM"))
                pps = cstk.enter_context(tc.tile_pool(name="ppsum", bufs=1,
                                                      space="PSUM"))

                IM = cpool.tile([81, BL * 400], f32, name="im", tag="im")
                CW = cpool.tile([81, 256], f32, name="cw", tag="cw")
                CB = cpool.tile([128, 2], f32, name="cb", tag="cb")
                PB = cpool.tile([128, 2], f32, name="pb", tag="pb")
                nc.sync.dma_start(out=IM[:], in_=im2_d[:])
                nc.sync.dma_start(out=CW[:], in_=convw_d[:])
                nc.sync.dma_start(out=CB[:], in_=convb_d[:])
                nc.sync.dma_start(out=PB[:], in_=primb_d[:])

                # SPIKES[kc] holds conv spikes for all (t, b): [128, 8000]
                SPIKES = [cpool.tile([128, T * BL * 400], bf16, name=f"spk{kc}", tag=f"spk{kc}")
                          for kc in range(2)]
                CONVOUT = [cpool.tile([128, BL * 400], f32, name=f"co{kc}", tag=f"co{kc}")
                           for kc in range(2)]
                MPC = [cpool.tile([128, BL * 400], f32, name=f"mpc{kc}", tag=f"mpc{kc}")
                       for kc in range(2)]
                ASC = [cpool.tile([128, BL * 400], f32, name=f"asc{kc}", tag=f"asc{kc}")
                       for kc in range(2)]

                # --- conv1: out[co, (b,pix)] = sum_k convw[k,co] im2[k,(b,pix)]
                for mc in range(2):
                    for b in range(BL):
                        pc = cps.tile([128, 400], f32, name="cvp", tag="cvp")
                        nc.tensor.matmul(
                            out=pc[:, 0:400],
                            lhsT=CW[:, mc * 128:(mc + 1) * 128],
                            rhs=IM[:, b * 400:(b + 1) * 400],
                            start=True, stop=True)
                        # relu(x + bias): exact via DVE tensor_scalar chain
                        nc.vector.tensor_scalar(
                            out=CONVOUT[mc][:, b * 400:(b + 1) * 400],
                            in0=pc[:, 0:400],
                            scalar1=CB[:, mc:mc + 1], scalar2=0.0,
                            op0=Alu.add, op1=Alu.max)

                # --- conv membrane dynamics for all T steps ---
                for t in range(T):
                    for kc in range(2):
                        spk = A(SPIKES[kc], 0, 128, [(1, BL * 400)],
                                t * BL * 400)
                        if t == 0:
                            nc.vector.tensor_scalar(
                                out=spk, in0=CONVOUT[kc][:],
                                scalar1=1.0, scalar2=None, op0=Alu.is_gt)
                            nc.gpsimd.tensor_tensor(
                                out=MPC[kc][:], in0=CONVOUT[kc][:], in1=spk,
                                op=Alu.subtract)
                        else:
                            nc.vector.scalar_tensor_tensor(
                                out=ASC[kc][:], in0=MPC[kc][:], scalar=0.2,
                                in1=CONVOUT[kc][:],
                                op0=Alu.mult, op1=Alu.add)
                            nc.vector.tensor_scalar(
                                out=spk, in0=ASC[kc][:],
                                scalar1=1.0, scalar2=None, op0=Alu.is_gt)
                            if t < T - 1:
                                nc.gpsimd.tensor_tensor(
                                    out=MPC[kc][:], in0=ASC[kc][:], in1=spk,
                                    op=Alu.subtract)

                if stage == "convmem":
                    nc.gpsimd.dma_start(out=D(outm_d, [(32, 128), (1, 32)]),
                                        in_=A(SPIKES[0], 0, 128, [(1, 32)]))
                    return
                # --- prim conv: batched over all (t, b):
                # out[co, (n=(t,b), oy, ox)] accumulated over (kc, ky, kx)
                PSP = [[pps.tile([128, 360], f32, name=f"pp{mc}{nch}", tag=f"pp{mc}{nch}")
                        for nch in range(2)] for mc in range(2)]
                for pos in range(81):
                    ky, kx = pos // 9, pos % 9
                    wt = wpool.tile([128, 512], bf16, name="w", tag="w")
                    (nc.sync if pos % 2 == 0 else nc.scalar).dma_start(
                        out=wt[:],
                        in_=D(primw_d, [(512, 128), (1, 512)], pos * 128 * 512))
                    for kc in range(2):
                        for mc in range(2):
                            lhsT = wt[:, kc * 256 + mc * 128:
                                      kc * 256 + (mc + 1) * 128]
                            for nch in range(2):
                                rhs = A(SPIKES[kc], 0, 128,
                                        [(400, 10), (40, 6), (2, 6)],
                                        nch * 4000 + ky * 20 + kx)
                                nc.tensor.matmul(
                                    out=PSP[mc][nch][:, 0:360],
                                    lhsT=lhsT, rhs=rhs,
                                    start=(pos == 0 and kc == 0),
                                    stop=(pos == 80 and kc == 1))

                # --- prim evac (+bias), membranes, spikes ---
                PRIM = [cpool.tile([128, T * BL * 36], f32, name=f"pr{mc}", tag=f"pr{mc}")
                        for mc in range(2)]
                PSPK = [cpool.tile([128, T * BL * 36], bf16, name=f"ps{mc}", tag=f"ps{mc}")
                        for mc in range(2)]
                MPP = [cpool.tile([128, BL * 36], f32, name=f"mpp{mc}", tag=f"mpp{mc}")
                       for mc in range(2)]
                APP = [cpool.tile([128, BL * 36], f32, name=f"app{mc}", tag=f"app{mc}")
                       for mc in range(2)]
                for mc in range(2):
                    for nch in range(2):
                        nc.vector.tensor_scalar(
                            out=PRIM[mc][:, nch * 360:(nch + 1) * 360],
                            in0=PSP[mc][nch][:, 0:360],
                            scalar1=PB[:, mc:mc + 1], scalar2=None,
                            op0=Alu.add)
                if stage == "prim":
                    nc.sync.dma_start(out=D(outm_d, [(32, 128), (1, 32)]),
                                      in_=A(PRIM[0], 0, 128, [(1, 32)]))
                    return
                for t in range(T):
                    for mc in range(2):
                        po = A(PRIM[mc], 0, 128, [(1, 144)], t * 144)
                        sp = A(PSPK[mc], 0, 128, [(1, 144)], t * 144)
                        if t == 0:
                            nc.vector.tensor_scalar(
                                out=sp, in0=po, scalar1=1.0, scalar2=None,
                                op0=Alu.is_gt)
                            nc.gpsimd.tensor_tensor(
                                out=MPP[mc][:], in0=po, in1=sp,
                                op=Alu.subtract)
                        else:
                            nc.vector.scalar_tensor_tensor(
                                out=APP[mc][:], in0=MPP[mc][:], scalar=0.2,
                                in1=po, op0=Alu.mult, op1=Alu.add)
                            nc.vector.tensor_scalar(
                                out=sp, in0=APP[mc][:], scalar1=1.0,
                                scalar2=None, op0=Alu.is_gt)
                            if t < T - 1:
                                nc.gpsimd.tensor_tensor(
                                    out=MPP[mc][:], in0=APP[mc][:], in1=sp,
                                    op=Alu.subtract)

                # --- prim spikes -> a2a_in ---
                # flat f = (mc*128+p)*36 + pix ; dest block j = f//1152,
                # rem = f%1152. Split partitions in groups of 32 so j is
                # constant per DMA:  p = ph*32 + pl ->
                # dst = j*23040 + b*5760 + t*1152 + pl*36 + pix, j = mc*4+ph
                env_spk0 = SPIKES[0]
                dmae = [nc.sync, nc.scalar, nc.gpsimd]
                di = 0
                for mc in range(2):
                    for ph in range(4):
                        j = mc * 4 + ph
                        for t in range(T):
                            src = A(PSPK[mc], ph * 32, 32,
                                    [(36, BL), (1, 36)], t * 144)
                            dst = D(a2a_in,
                                    [(36, 32), (5760, BL), (1, 36)],
                                    j * 23040 + t * 1152)
                            dmae[di % 3].dma_start(out=dst, in_=src)
                            di += 1

            if stage == "conv":
                nc.gpsimd.dma_start(out=D(outm_d, [(32, 128), (1, 32)]),
                                    in_=A(env_spk0, 0, 128, [(1, 32)]))
                return
            # =========== AllToAll: batch-shard -> route-shard ===========
            if solo:
                nc.sync.dma_start(out=a2a_out[:], in_=a2a_in[:])
            else:
                nc.gpsimd.collective_compute(
                    "AllToAll", Alu.bypass, replica_groups=rg,
                    ins=[a2a_in[:]], outs=[a2a_out[:]])

            if stage == "a2a":
                nc.gpsimd.dma_start(out=D(outm_d, [(32, 128), (1, 32)]),
                                    in_=D(a2a_out, [(32, 128), (1, 32)]))
                return
            # =========== Phase B prep: transposes + u_hat ===========
            # UH [128, T*5760]: col = t*5760 + rslot*32 + b.
            #   rslot 0..144: main co (partitions = co 0..128)
            #   rslot 144..180: tail co, partitions = (rq, cot) fold.
            # Digit bias (2x domain) is folded into the PSUM evac, so the
            # membrane threshold is a uniform 1.0 for all partitions/steps.
            NS = 5760
            uall = stk.enter_context(tc.tile_pool(name="uall", bufs=1))
            UH = uall.tile([128, T * NS], f32, name="uh", tag="uh")
            BIAV = uall.tile([128, 2], f32, name="biav", tag="biav")
            nc.sync.dma_start(out=BIAV[:], in_=biav_d[:])
            with ExitStack() as ustk:
                upool = ustk.enter_context(tc.tile_pool(name="uh", bufs=1))
                w2pool = ustk.enter_context(tc.tile_pool(name="w2c", bufs=3))
                ups = ustk.enter_context(tc.tile_pool(name="upsum", bufs=4,
                                                      space="PSUM"))

                IDT = upool.tile([32, 32], bf16, name="idt", tag="idt")
                from concourse.masks import make_identity
                make_identity(nc, IDT[:])
                X16 = [persist.tile([128, 160], bf16, name=f"x16_{g}",
                                    tag=f"x16_{g}") for g in range(9)]
                for g in range(9):
                    Mg = upool.tile([32, T * 128], bf16, name="mg", tag="mg",
                                    bufs=2)
                    nc.sync.dma_start(
                        out=Mg[:],
                        in_=D(a2a_out, [(5760, 32), (1152, T), (1, 128)],
                              g * 128))
                    for t in range(T):
                        pst = ups.tile([128, 32], bf16, name="pst", tag="pst", bufs=2)
                        nc.tensor.transpose(
                            out=pst[:], in_=Mg[:, t * 128:(t + 1) * 128],
                            identity=IDT[:])
                        nc.vector.tensor_copy(
                            out=X16[g][:, t * 32:(t + 1) * 32], in_=pst[:])

                if stage == "trans":
                    nc.gpsimd.dma_start(
                        out=D(outm_d, [(32, 128), (1, 32)]),
                        in_=A(X16[0], 0, 128, [(1, 32)]))
                    return
                ActF = mybir.ActivationFunctionType
                for g in range(9):
                    w2c = w2pool.tile([128, 16 * CO], bf16, name="w2c",
                                      tag="w2c", bufs=2)
                    (nc.sync if g % 2 == 0 else nc.scalar).dma_start(
                        out=w2c[:],
                        in_=D(w2g_d, [(2560, 128), (1, 2560)],
                              g * 128 * 2560))
                    for r2 in range(8):
                        psA = ups.tile([128, 320], f32, name="upa", tag="upa",
                                       bufs=3)
                        psB = ups.tile([128, 320], f32, name="upb", tag="upb",
                                       bufs=3)
                        for j in range(2):
                            rr = r2 * 2 + j
                            r = g * 16 + rr
                            rq = r // 36
                            q = (rr // 4) * 32
                            rhs = A(X16[g], q, 32, [(1, 160)])
                            nc.tensor.matmul(
                                out=psA[:, j * 160:(j + 1) * 160],
                                lhsT=A(w2c, q, 32, [(1, 128)], rr * CO),
                                rhs=rhs, start=True, stop=True,
                                tile_position=(q, 0))
                            nc.tensor.matmul(
                                out=A(psB, rq * 32, 32, [(1, 160)], j * 160),
                                lhsT=A(w2c, q, 32, [(1, 32)], rr * CO + 128),
                                rhs=rhs, start=True, stop=True,
                                tile_position=(q, rq * 32))
                        r0 = g * 16 + r2 * 2
                        rq0, rl0 = r0 // 36, r0 % 36
                        # evac + per-partition bias add (Act for main on
                        # 128 parts, DVE for the tail band).
                        nc.scalar.activation(
                            out=A(UH, 0, 128, [(32, 2), (NS, T), (1, 32)],
                                  r0 * 32),
                            in_=A(psA, 0, 128, [(160, 2), (32, T), (1, 32)]),
                            func=ActF.Identity,
                            bias=BIAV[:, 0:1], scale=1.0)
                        nc.vector.tensor_scalar(
                            out=A(UH, rq0 * 32, 32,
                                  [(32, 2), (NS, T), (1, 32)],
                                  (144 + rl0) * 32),
                            in0=A(psB, rq0 * 32, 32,
                                  [(160, 2), (32, T), (1, 32)]),
                            scalar1=A(BIAV, rq0 * 32, 32, [(1, 1)], 1),
                            scalar2=None, op0=Alu.add)

            if stage == "uhat":
                nc.gpsimd.dma_start(out=D(outm_d, [(32, 128), (1, 32)]),
                                    in_=A(X16[0], 0, 128, [(1, 32)]))
                return
            # =========== Phase B: digit-caps loop (route-sharded) ========
            dpool = stk.enter_context(tc.tile_pool(name="dig", bufs=1))
            dps = stk.enter_context(tc.tile_pool(name="dpsum", bufs=1,
                                                 space="PSUM"))

            SELAb = dpool.tile([128, 160], bf16, name="sela", tag="sela")
            SELTb = dpool.tile([32, 160], bf16, name="selt", tag="selt")
            REPL = dpool.tile([32, 128], bf16, name="repl", tag="repl")
            FOLD = dpool.tile([128, 32], f32, name="fold", tag="fold")
            nc.sync.dma_start(out=SELAb[:], in_=sela_d[:])
            nc.sync.dma_start(out=SELTb[:], in_=selt_d[:])
            nc.sync.dma_start(out=REPL[:], in_=repl_d[:])
            nc.sync.dma_start(out=FOLD[:], in_=fold_d[:])
            IDT128 = dpool.tile([128, 128], f32, name="idt128", tag="idt128")
            from concourse.masks import make_identity as _mkid
            _mkid(nc, IDT128[:])

            DSB = [dpool.tile([128, NS], bf16, name=f"dsb{i}", tag=f"dsb{i}")
                   for i in range(2)]
            TR = dpool.tile([128, NS], bf16, name="tr", tag="tr")
            Y = dpool.tile([128, NS], bf16, name="y", tag="y")
            Z = dpool.tile([128, NS], bf16, name="z", tag="z")
            BIJf = dpool.tile([128, 180], f32, name="bijf", tag="bijf")
            BIJb = dpool.tile([128, 180], bf16, name="bijb", tag="bijb")
            SJF = dpool.tile([128, 32], f32, name="sjf", tag="sjf")
            SJQ = dpool.tile([128, 32], f32, name="sjq", tag="sjq")
            SJG_m = dpool.tile([128, 32], f32, name="sjgm", tag="sjgm")
            SJG_t = dpool.tile([32, 32], f32, name="sjgt", tag="sjgt")
            SJT = dpool.tile([32, 32], f32, name="sjt", tag="sjt")
            PDTS = dpool.tile([32, 145], f32, name="pdts", tag="pdts")
            OUT_m = dpool.tile([128, 32], f32, name="outm", tag="outm")
            OUT_t = dpool.tile([32, 32], f32, name="outt", tag="outt")
            A2_m = dpool.tile([128, 32], f32, name="a2m", tag="a2m")
            A2_t = dpool.tile([32, 32], f32, name="a2t", tag="a2t")
            M2_m = dpool.tile([128, 32], f32, name="m2m", tag="m2m")
            M2_t = dpool.tile([32, 32], f32, name="m2t", tag="m2t")
            D2m = dpool.tile([128, 32], bf16, name="d2m", tag="d2m")
            D2t = dpool.tile([32, 32], bf16, name="d2t", tag="d2t")
            D2F = dpool.tile([128, 32], bf16, name="d2f", tag="d2f")
            SCR_m = dpool.tile([128, 32], f32, name="scrm", tag="scrm")
            SCR_t = dpool.tile([32, 32], f32, name="scrt", tag="scrt")
            DSU_m = dpool.tile([128, 1], f32, name="dsum", tag="dsum")
            DSU_t = dpool.tile([32, 1], f32, name="dsut", tag="dsut")
            DSUbm = dpool.tile([128, 1], bf16, name="dsubm", tag="dsubm")
            DSUbt = dpool.tile([32, 1], bf16, name="dsubt", tag="dsubt")
            ZBt = dpool.tile([128, 36], f32, name="zbt", tag="zbt")
            ZBT4 = dpool.tile([32, 144], bf16, name="zbt4", tag="zbt4")
            DPDF = dpool.tile([128, 37], f32, name="dpdf", tag="dpdf")
            PDSm = dpool.tile([128, 1], f32, name="pdsm", tag="pdsm")

            bij0 = float(np.float32(1.0) / np.float32(R))
            nc.vector.memset(BIJf[:], bij0)
            nc.vector.tensor_copy(out=BIJb[:], in_=BIJf[:])

            # AP views
            def Uf(t):
                return A(UH, 0, 128, [(1, NS)], t * NS)

            def Ur(t, s0=0, cnt=180):
                return A(UH, 0, 128, [(32, cnt), (1, 32)], t * NS + s0 * 32)

            def DSr(i):   # (rslot, b) iteration over (b,r)-major storage
                return A(DSB[i], 0, 128, [(1, 180), (180, 32)])

            def DSbb(i):  # (b, rslot) iteration
                return A(DSB[i], 0, 128, [(180, 32), (1, 180)])

            TRr = A(TR, 0, 128, [(32, 180), (1, 32)])

            # prologue: spikes at t=0 (m0 = u0), trace init = ds0
            nc.vector.tensor_scalar(out=DSr(0), in0=Ur(0), scalar1=1.0,
                                    scalar2=None, op0=Alu.is_gt)
            nc.gpsimd.tensor_tensor(out=Ur(0), in0=Ur(0), in1=DSr(0),
                                    op=Alu.subtract)
            nc.scalar.copy(out=TR[:], in_=DSB[0][:])

            for t in range(T):
                i = t % 2
                # ---- y = ds * bij ; s_j partials; local OUT accum ----
                nc.vector.tensor_tensor(
                    out=A(Y, 0, 128, [(180, 32), (1, 180)]),
                    in0=DSbb(i),
                    in1=A(BIJb, 0, 128, [(0, 32), (1, 180)]),
                    op=Alu.mult)
                nc.vector.tensor_reduce(
                    out=SJF[:], in_=A(Y, 0, 128, [(180, 32), (1, 144)]),
                    axis=mybir.AxisListType.X, op=Alu.add)
                nc.vector.tensor_reduce(
                    out=SJQ[:], in_=A(Y, 0, 128, [(180, 32), (1, 36)], 144),
                    axis=mybir.AxisListType.X, op=Alu.add)
                SJTps = dps.tile([32, 32], f32, name="sjtp", tag="sjtp")
                nc.tensor.matmul(out=SJTps[:], lhsT=FOLD[:], rhs=SJQ[:],
                                 start=True, stop=True)
                nc.vector.tensor_copy(out=SJT[:], in_=SJTps[:])
                if t == 0:
                    nc.vector.tensor_copy(out=OUT_m[:], in_=SJF[:])
                    nc.vector.tensor_copy(out=OUT_t[:], in_=SJT[:])
                else:
                    nc.vector.tensor_tensor(out=OUT_m[:], in0=OUT_m[:],
                                            in1=SJF[:], op=Alu.add)
                    nc.vector.tensor_tensor(out=OUT_t[:], in0=OUT_t[:],
                                            in1=SJT[:], op=Alu.add)
                if t == T - 1:
                    break
                # ---- exchange s_j partials (AllReduce) ----
                nc.sync.dma_start(out=D(sj_in[t], [(32, 128), (1, 32)]),
                                  in_=SJF[:])
                nc.sync.dma_start(out=D(sj_in[t], [(32, 32), (1, 32)], 4096),
                                  in_=SJT[:])
                if solo:
                    nc.sync.dma_start(out=sj_out[t][:], in_=sj_in[t][:])
                else:
                    nc.gpsimd.collective_compute(
                        "AllReduce", Alu.add, replica_groups=rg,
                        ins=[sj_in[t][:]], outs=[sj_out[t][:]])

                # ---- membranes for t+1 (overlaps the collective) ----
                i2 = (t + 1) % 2
                nc.vector.scalar_tensor_tensor(
                    out=Uf(t + 1), in0=Uf(t), scalar=0.2, in1=Uf(t + 1),
                    op0=Alu.mult, op1=Alu.add)
                nc.vector.tensor_scalar(out=DSr(i2), in0=Ur(t + 1),
                                        scalar1=1.0, scalar2=None,
                                        op0=Alu.is_gt)
                if t + 1 < T - 1:
                    nc.gpsimd.tensor_tensor(out=Ur(t + 1), in0=Ur(t + 1),
                                            in1=DSr(i2), op=Alu.subtract)

                # ---- consume AllReduce t: dig2 chain ----
                nc.sync.dma_start(out=SJG_m[:],
                                  in_=D(sj_out[t], [(32, 128), (1, 32)]))
                nc.sync.dma_start(out=SJG_t[:],
                                  in_=D(sj_out[t], [(32, 32), (1, 32)], 4096))
                if t == 0:
                    a2m, a2t = SJG_m, SJG_t
                else:
                    nc.vector.scalar_tensor_tensor(
                        out=A2_m[:], in0=M2_m[:], scalar=0.2, in1=SJG_m[:],
                        op0=Alu.mult, op1=Alu.add)
                    nc.vector.scalar_tensor_tensor(
                        out=A2_t[:], in0=M2_t[:], scalar=0.2, in1=SJG_t[:],
                        op0=Alu.mult, op1=Alu.add)
                    a2m, a2t = A2_m, A2_t
                nc.vector.tensor_scalar(out=D2m[:], in0=a2m[:], scalar1=0.5,
                                        scalar2=None, op0=Alu.is_gt)
                nc.vector.tensor_scalar(out=D2t[:], in0=a2t[:], scalar1=0.5,
                                        scalar2=None, op0=Alu.is_gt)
                if t < T - 2:
                    nc.vector.scalar_tensor_tensor(
                        out=M2_m[:], in0=D2m[:], scalar=-0.5, in1=a2m[:],
                        op0=Alu.mult, op1=Alu.add)
                    nc.vector.scalar_tensor_tensor(
                        out=M2_t[:], in0=D2t[:], scalar=-0.5, in1=a2t[:],
                        op0=Alu.mult, op1=Alu.add)
                # replicate tail d2s to the (rq,cot) fold layout via PE
                REPps = dps.tile([128, 32], f32, name="repp", tag="repp")
                nc.tensor.matmul(out=REPps[:], lhsT=REPL[:], rhs=D2t[:],
                                 start=True, stop=True)
                nc.vector.tensor_copy(out=D2F[:], in_=REPps[:])
                # -0.1 * alpha * sum_b d2s terms
                nc.vector.tensor_scalar(
                    out=SCR_m[:], in0=D2m[:],
                    scalar1=float(np.float32(0.1) * ALPHA), scalar2=None,
                    op0=Alu.mult, op1=Alu.add, accum_out=DSU_m[:])
                nc.vector.tensor_scalar(
                    out=SCR_t[:], in0=D2t[:],
                    scalar1=float(np.float32(0.1) * ALPHA), scalar2=None,
                    op0=Alu.mult, op1=Alu.add, accum_out=DSU_t[:])
                nc.vector.tensor_copy(out=DSUbm[:], in_=DSU_m[:])
                nc.vector.tensor_copy(out=DSUbt[:], in_=DSU_t[:])

                # ---- z = trace * d2s ; tail zb ----
                nc.vector.tensor_tensor(
                    out=A(Z, 0, 128, [(32, 144), (1, 32)]),
                    in0=A(TR, 0, 128, [(32, 144), (1, 32)]),
                    in1=A(D2m, 0, 128, [(0, 144), (1, 32)]),
                    op=Alu.mult)
                nc.vector.tensor_tensor(
                    out=A(Z, 0, 128, [(32, 36), (1, 32)], 144 * 32),
                    in0=A(TR, 0, 128, [(32, 36), (1, 32)], 144 * 32),
                    in1=A(D2F, 0, 128, [(0, 36), (1, 32)]),
                    op=Alu.mult)
                nc.vector.tensor_reduce(
                    out=ZBt[:, 0:36],
                    in_=A(Z, 0, 128, [(32, 36), (1, 32)], 144 * 32),
                    axis=mybir.AxisListType.X, op=Alu.add)
                # fold ZBt [128,36] -> ZBT4 [32, (rq,36)] via PE
                ZBT4ps = dps.tile([32, 144], f32, name="zbt4p", tag="zbt4p")
                for rq in range(4):
                    nc.tensor.matmul(
                        out=A(ZBT4ps, 0, 32, [(1, 36)], rq * 36),
                        lhsT=IDT128[:, rq * 32:(rq + 1) * 32],
                        rhs=ZBt[:, 0:36],
                        start=(rq == 0), stop=(rq == 3))
                nc.vector.tensor_copy(out=ZBT4[:], in_=ZBT4ps[:])

                # ---- delta matmuls: PE accumulates over b ----
                PD_m = dps.tile([128, 144], f32, name="pdm", tag="pdm")
                PD_t = dps.tile([32, 144], f32, name="pdt", tag="pdt")
                PDm1 = dps.tile([128, 1], f32, name="pdm1", tag="pdm1")
                PDt1 = dps.tile([32, 1], f32, name="pdt1", tag="pdt1")
                for b in range(32):
                    nc.tensor.matmul(out=PD_m[:, 0:144],
                                     lhsT=SELAb[:, 0:128],
                                     rhs=A(Z, 0, 128, [(32, 144)], b),
                                     start=(b == 0), stop=False)
                for b in range(32):
                    nc.tensor.matmul(out=PD_t[:, 0:144],
                                     lhsT=SELAb[:, 128:160],
                                     rhs=A(Z, 0, 128, [(32, 144)], b),
                                     start=(b == 0), stop=False)
                nc.tensor.matmul(out=PD_m[:, 0:144], lhsT=SELTb[:, 0:128],
                                 rhs=ZBT4[:], start=False, stop=True)
                nc.tensor.matmul(out=PD_t[:, 0:144], lhsT=SELTb[:, 128:160],
                                 rhs=ZBT4[:], start=False, stop=True)
                nc.tensor.matmul(out=PDm1[:], lhsT=SELAb[:, 0:128],
                                 rhs=DSUbm[:], start=True, stop=False)
                nc.tensor.matmul(out=PDm1[:], lhsT=SELTb[:, 0:128],
                                 rhs=DSUbt[:], start=False, stop=True)
                nc.tensor.matmul(out=PDt1[:], lhsT=SELAb[:, 128:160],
                                 rhs=DSUbm[:], start=True, stop=False)
                nc.tensor.matmul(out=PDt1[:], lhsT=SELTb[:, 128:160],
                                 rhs=DSUbt[:], start=False, stop=True)

                # ---- bij updates ----
                nc.vector.scalar_tensor_tensor(
                    out=A(BIJf, 0, 128, [(1, 144)]),
                    in0=PD_m[:, 0:144], scalar=float(ALPHA),
                    in1=A(BIJf, 0, 128, [(1, 144)]),
                    op0=Alu.mult, op1=Alu.add)
                nc.vector.tensor_copy(out=PDSm[:], in_=PDm1[:])
                nc.vector.tensor_scalar(
                    out=A(BIJf, 0, 128, [(1, 144)]),
                    in0=A(BIJf, 0, 128, [(1, 144)]),
                    scalar1=PDSm[:, 0:1], scalar2=None, op0=Alu.subtract)
                # tail: partition-expand PD_t/PDt1 via small DMAs
                nc.vector.tensor_copy(out=PDTS[:, 0:144], in_=PD_t[:])
                nc.vector.tensor_copy(out=PDTS[:, 144:145], in_=PDt1[:])
                dmaq = [nc.sync, nc.scalar]
                for rq in range(4):
                    dmaq[rq % 2].dma_start(
                        out=A(DPDF, rq * 32, 32, [(1, 36)]),
                        in_=A(PDTS, 0, 32, [(1, 36)], rq * 36))
                    dmaq[rq % 2].dma_start(
                        out=A(DPDF, rq * 32, 32, [(1, 1)], 36),
                        in_=PDTS[:, 144:145])
                nc.vector.scalar_tensor_tensor(
                    out=A(BIJf, 0, 128, [(1, 36)], 144),
                    in0=DPDF[:, 0:36], scalar=float(ALPHA),
                    in1=A(BIJf, 0, 128, [(1, 36)], 144),
                    op0=Alu.mult, op1=Alu.add)
                nc.vector.tensor_scalar(
                    out=A(BIJf, 0, 128, [(1, 36)], 144),
                    in0=A(BIJf, 0, 128, [(1, 36)], 144),
                    scalar1=DPDF[:, 36:37], scalar2=None, op0=Alu.subtract)
                nc.vector.tensor_copy(out=BIJb[:], in_=BIJf[:])

                # ---- trace update for t+1 (after z(t) consumed TR) ----
                # Pool rejects scalar_tensor_tensor on TRN2; split into a
                # DVE 4x tensor_scalar (into Z scratch) + Pool max.
                if t < T - 2:
                    nc.vector.tensor_scalar(
                        out=Z[:], in0=TR[:], scalar1=float(DECAY_TR),
                        scalar2=None, op0=Alu.mult)
                    nc.vector.tensor_tensor(
                        out=A(TR, 0, 128, [(32, 180), (1, 32)]),
                        in0=A(Z, 0, 128, [(32, 180), (1, 32)]),
                        in1=DSr(i2), op=Alu.max)

            # ---- write outputs (local partial sums; host adds cores) ----
            nc.sync.dma_start(out=D(outm_d, [(32, 128), (1, 32)]),
                              in_=OUT_m[:])
            nc.sync.dma_start(out=D(outm_d, [(32, 32), (1, 32)], 128 * 32),
                              in_=OUT_t[:])
            nc.sync.dma_start(out=dbg_d[:], in_=A(UH, 0, 128, [(1, 512)]))


def _host_prepare(data, conv_w, conv_b, prim_w, prim_b, W, bias):
    """Build per-core input maps."""
    from numpy.lib.stride_tricks import sliding_window_view
    f32 = np.float32
    data = np.asarray(data, f32)
    conv_w = np.asarray(conv_w, f32)
    conv_b = np.asarray(conv_b, f32)
    prim_w = np.asarray(prim_w, f32)
    prim_b = np.asarray(prim_b, f32)
    W = np.asarray(W, f32)
    bias = np.asarray(bias, f32)

    # im2col: win[b, ky, kx, oy, ox]
    win = sliding_window_view(data[:, 0, :, :], (20, 20), axis=(1, 2))
    im2_all = np.ascontiguousarray(win).reshape(B, 81, 400)

    # everything feeding the spiking membranes runs in a 2x-scaled domain
    # (exact in fp32) so the reset is the plain subtract M = A - ds.
    convw = np.ascontiguousarray(conv_w.reshape(256, 81).T) * f32(2.0)
    convb2 = np.ascontiguousarray(conv_b.reshape(2, 128).T) * f32(2.0)

    import ml_dtypes
    bf16 = ml_dtypes.bfloat16
    pw = prim_w.reshape(2, 128, 2, 128, 9, 9)
    primw = (np.ascontiguousarray(
        pw.transpose(4, 5, 3, 2, 0, 1).reshape(81, 128, 512))
        * f32(2.0)).astype(bf16)
    primb2 = np.ascontiguousarray(prim_b.reshape(2, 128).T) * f32(2.0)

    # W2[i, r, co] with co = o*10 + c, zero-padded to K=32 route-quads:
    # w2g[g, rr*8+i, rr*160+co] = 2*W2[i, g*16+rr, co]
    Wt = np.ascontiguousarray(
        W.transpose(3, 0, 2, 1)).reshape(8, R, CO) * f32(2.0)

    # per-partition digit bias (2x domain), col0 = main co, col1 = tail fold
    bias_o = bias[:, 0]
    biav = np.zeros((128, 2), f32)
    for p in range(128):
        biav[p, 0] = f32(2.0) * bias_o[p // 10]
        biav[p, 1] = f32(2.0) * bias_o[(128 + p % 32) // 10]

    cos = np.arange(CO)
    sela = np.ascontiguousarray(
        (np.equal.outer(cos[:128] % 10, cos % 10)).astype(bf16))
    selt = np.ascontiguousarray(
        (np.equal.outer(cos[128:] % 10, cos % 10)).astype(bf16))
    # repl[k, m] = [k == m % 32]  (replicate [32,x] -> [128,x] via PE)
    repl = np.ascontiguousarray(
        np.equal.outer(np.arange(32), np.arange(128) % 32).astype(bf16))
    # fold[p, m] = [p % 32 == m]  (sum 4 rq-groups of partitions)
    fold = np.ascontiguousarray(
        np.equal.outer(np.arange(128) % 32, np.arange(32)).astype(f32))

    in_maps = []
    for k in range(N_CORES):
        im2 = np.ascontiguousarray(
            im2_all[BL * k:BL * (k + 1)].transpose(1, 0, 2).reshape(81, 1600))
        w2core = Wt[:, RL * k:RL * (k + 1), :]          # [8, 144, 160]
        w2g = np.zeros((9, 128, 16 * CO), bf16)
        for rr in range(16):
            # [8, 9, 160] block for this rr across all 9 groups
            blk = w2core[:, rr::16, :]
            w2g[:, rr * 8:(rr + 1) * 8, rr * CO:(rr + 1) * CO] = \
                blk.transpose(1, 0, 2)
        in_maps.append({
            "im2": im2, "convw": convw, "convb": convb2,
            "primw": primw, "primb": primb2, "w2g": w2g,
            "biav": biav, "sela": sela, "selt": selt,
            "repl": repl, "fold": fold,
        })
    return in_maps


HOST_SUM_OUT = True


def _postprocess(outm):
    """outm [160, 32] (co = o*10+c) -> classes [32, 10]."""
    out3 = outm.reshape(16, 10, 32).astype(np.float32) / np.float32(T)
    sq = (out3 * out3).sum(axis=0)
    return np.sqrt(sq).T.astype(np.float32)


def kernel(data, conv_w, conv_b, prim_w, prim_b, W, bias, time_window):
    from concourse.bass_utils import run_bass_kernel_spmd
    assert int(time_window) == T
    if "nc" not in _CACHE:
        _CACHE["nc"] = _build_program()
    nc = _CACHE["nc"]
    in_maps = _host_prepare(data, conv_w, conv_b, prim_w, prim_b, W, bias)
    res = run_bass_kernel_spmd(nc, in_maps, core_ids=list(range(N_CORES)))
    outm = np.sum([np.asarray(res.results[k]["outm"], np.float32)
                   for k in range(N_CORES)], axis=0)
    return _postprocess(outm)



# revision 22
# speedup vs baseline: 1.0307x; 1.0307x over previous
"""Trainium2 Bass kernel for the spiking CapsNet forward pass (nn_CapsNet).

Strategy (8 NeuronCores):
  Phase A (batch-parallel, 4 images/core):
    conv1 once (input is constant over the 5 timesteps), conv-layer membrane
    dynamics for all 5 steps upfront, then the expensive prim conv batched
    over all 5 timesteps in one weight-stationary PE pass.
  AllToAll: re-shard prim spikes from batch-split to route-split.
  Phase B (route-parallel, 144 routes/core, full batch):
    u_hat on PE per route (contraction over i=8), digit-caps membrane /
    trace / routing chain on DVE+GPSIMD with a small AllGather of the
    per-core s_j partials each step.
Host side: input re-layout (im2col, weight transposes) and the final
  classes = sqrt(sum_o (out_mem/T)^2) reduction.
"""

import numpy as np

N_CORES = 8
T = 5
B = 32
BL = B // N_CORES          # local batch (4)
R = 1152
RL = R // N_CORES          # local routes (144)
CO = 160                   # (o,c) pairs, ordered co = o*10 + c
DECAY = np.float32(0.2)
THRESH = np.float32(0.5)
DECAY_TR = np.float32(np.exp(np.float32(-1.0 / 1.5)))
ALPHA = np.float32(np.float32(0.0008) / np.float32(32.0))

_CACHE = {}


def _build_program(reps=1, stage="full", solo=False):
    import concourse.bass as bass
    import concourse.mybir as mybir
    import concourse.tile as tile
    from concourse import bacc

    Alu = mybir.AluOpType
    Act = mybir.ActivationFunctionType
    f32 = mybir.dt.float32
    bf16 = mybir.dt.bfloat16

    nc = bacc.Bacc("TRN2", target_bir_lowering=False, debug=False,
                   num_devices=N_CORES)

    # ---- external I/O (per-core values supplied by host) ----
    im2_d = nc.dram_tensor("im2", [81, BL * 400], f32, kind="ExternalInput")
    convw_d = nc.dram_tensor("convw", [81, 256], f32, kind="ExternalInput")
    convb_d = nc.dram_tensor("convb", [128, 2], f32, kind="ExternalInput")
    primw_d = nc.dram_tensor("primw", [81, 128, 512], bf16,
                             kind="ExternalInput")
    primb_d = nc.dram_tensor("primb", [128, 2], f32, kind="ExternalInput")
    w2g_d = nc.dram_tensor("w2g", [9, 128, 16 * CO], bf16,
                           kind="ExternalInput")
    biav_d = nc.dram_tensor("biav", [128, 2], f32, kind="ExternalInput")
    sela_d = nc.dram_tensor("sela", [128, 160], bf16, kind="ExternalInput")
    selt_d = nc.dram_tensor("selt", [32, 160], bf16, kind="ExternalInput")
    repl_d = nc.dram_tensor("repl", [32, 128], bf16, kind="ExternalInput")
    fold_d = nc.dram_tensor("fold", [128, 32], f32, kind="ExternalInput")
    outm_d = nc.dram_tensor("outm", [160, 32], f32, kind="ExternalOutput")
    dbg_d = nc.dram_tensor("dbg", [128, 512], f32, kind="ExternalOutput")

    # ---- internal DRAM ----
    # a2a: per-dest-rank blocks [dest 8][b_l 4][t 5][flat 1152]
    a2a_in = nc.dram_tensor("a2a_in", [N_CORES * BL * T * 1152], bf16)
    a2a_out = nc.dram_tensor("a2a_out", [N_CORES * BL * T * 1152], bf16)
    # per-step s_j exchange: AllGather of 20KB partials + local 8-way sum
    SJP = 5120
    sj_in = [nc.dram_tensor(f"sj_in{t}", [SJP], f32) for t in range(T - 1)]
    sj_out = [nc.dram_tensor(f"sj_out{t}", [N_CORES * SJP], f32,
                             addr_space="Shared") for t in range(T - 1)]

    def A(t, p0, pc, dims, foff=0):
        """Raw AP on tile/tensor t: partitions [p0, p0+pc), free dims list
        [(step, count), ...] in elements, plus extra free offset."""
        b = t if isinstance(t, bass.AP) else t[:]
        pitch = b.ap[0][0]
        return bass.AP(b.tensor, b.offset + p0 * pitch + foff,
                       [[pitch, pc]] + [list(d) for d in dims])

    def D(h, dims, off=0):
        """Raw AP on a DRAM tensor handle (flat element space)."""
        b = h[:]
        return bass.AP(b.tensor, off, [list(d) for d in dims])

    rg = [list(range(N_CORES))]

    with tile.TileContext(nc) as tc:
        from contextlib import ExitStack
        for _rep in range(reps):
            _run_once(nc, tc, bass, mybir, locals(), stage, solo)

    nc.compile()
    return nc


def _run_once(nc, tc, bass, mybir, env, stage="full", solo=False):
    import numpy as np
    from contextlib import ExitStack
    Alu = mybir.AluOpType
    f32 = mybir.dt.float32
    bf16 = mybir.dt.bfloat16
    im2_d = env["im2_d"]; convw_d = env["convw_d"]; convb_d = env["convb_d"]
    primw_d = env["primw_d"]; primb_d = env["primb_d"]
    biav_d = env["biav_d"]; sela_d = env["sela_d"]
    repl_d = env["repl_d"]; fold_d = env["fold_d"]
    selt_d = env["selt_d"]; outm_d = env["outm_d"]; dbg_d = env["dbg_d"]
    a2a_in = env["a2a_in"]; a2a_out = env["a2a_out"]
    w2g_d = env["w2g_d"]
    sj_in = env["sj_in"]; sj_out = env["sj_out"]; SJP = env["SJP"]
    A = env["A"]; D = env["D"]; rg = env["rg"]

    if True:
        with ExitStack() as stk:
            # ---------------- persistent pools ----------------
            persist = stk.enter_context(tc.tile_pool(name="persist", bufs=1))
            dram = stk.enter_context(tc.tile_pool(name="dram", bufs=1,
                                                  space="DRAM"))

            # =========== Phase A: conv stage (batch-sharded) ===========
            with ExitStack() as cstk:
                cpool = cstk.enter_context(tc.tile_pool(name="conv", bufs=1))
                wpool = cstk.enter_context(tc.tile_pool(name="wpos", bufs=4))
                cps = cstk.enter_context(tc.tile_pool(name="cpsum", bufs=4,
                                                      space="PSUM"))
                pps = cstk.enter_context(tc.tile_pool(name="ppsum", bufs=1,
                                                      space="PSUM"))

                IM = cpool.tile([81, BL * 400], f32, name="im", tag="im")
                CW = cpool.tile([81, 256], f32, name="cw", tag="cw")
                CB = cpool.tile([128, 2], f32, name="cb", tag="cb")
                PB = cpool.tile([128, 2], f32, name="pb", tag="pb")
                nc.sync.dma_start(out=IM[:], in_=im2_d[:])
                nc.sync.dma_start(out=CW[:], in_=convw_d[:])
                nc.sync.dma_start(out=CB[:], in_=convb_d[:])
                nc.sync.dma_start(out=PB[:], in_=primb_d[:])

                # SPIKES[kc] holds conv spikes for all (t, b): [128, 8000]
                SPIKES = [cpool.tile([128, T * BL * 400], bf16, name=f"spk{kc}", tag=f"spk{kc}")
                          for kc in range(2)]
                CONVOUT = [cpool.tile([128, BL * 400], f32, name=f"co{kc}", tag=f"co{kc}")
                           for kc in range(2)]
                MPC = [cpool.tile([128, BL * 400], f32, name=f"mpc{kc}", tag=f"mpc{kc}")
                       for kc in range(2)]
                ASC = [cpool.tile([128, BL * 400], f32, name=f"asc{kc}", tag=f"asc{kc}")
                       for kc in range(2)]

                # --- conv1: out[co, (b,pix)] = sum_k convw[k,co] im2[k,(b,pix)]
                for mc in range(2):
                    for b in range(BL):
                        pc = cps.tile([128, 400], f32, name="cvp", tag="cvp")
                        nc.tensor.matmul(
                            out=pc[:, 0:400],
                            lhsT=CW[:, mc * 128:(mc + 1) * 128],
                            rhs=IM[:, b * 400:(b + 1) * 400],
                            start=True, stop=True)
                        # relu(x + bias): exact via DVE tensor_scalar chain
                        nc.vector.tensor_scalar(
                            out=CONVOUT[mc][:, b * 400:(b + 1) * 400],
                            in0=pc[:, 0:400],
                            scalar1=CB[:, mc:mc + 1], scalar2=0.0,
                            op0=Alu.add, op1=Alu.max)

                # --- conv membrane dynamics for all T steps ---
                for t in range(T):
                    for kc in range(2):
                        spk = A(SPIKES[kc], 0, 128, [(1, BL * 400)],
                                t * BL * 400)
                        if t == 0:
                            nc.vector.tensor_scalar(
                                out=spk, in0=CONVOUT[kc][:],
                                scalar1=1.0, scalar2=None, op0=Alu.is_gt)
                            nc.gpsimd.tensor_tensor(
                                out=MPC[kc][:], in0=CONVOUT[kc][:], in1=spk,
                                op=Alu.subtract)
                        else:
                            nc.vector.scalar_tensor_tensor(
                                out=ASC[kc][:], in0=MPC[kc][:], scalar=0.2,
                                in1=CONVOUT[kc][:],
                                op0=Alu.mult, op1=Alu.add)
                            nc.vector.tensor_scalar(
                                out=spk, in0=ASC[kc][:],
                                scalar1=1.0, scalar2=None, op0=Alu.is_gt)
                            if t < T - 1:
                                nc.gpsimd.tensor_tensor(
                                    out=MPC[kc][:], in0=ASC[kc][:], in1=spk,
                                    op=Alu.subtract)

                if stage == "convmem":
                    nc.gpsimd.dma_start(out=D(outm_d, [(32, 128), (1, 32)]),
                                        in_=A(SPIKES[0], 0, 128, [(1, 32)]))
                    return
                # --- prim conv: batched over all (t, b):
                # out[co, (n=(t,b), oy, ox)] accumulated over (kc, ky, kx)
                PSP = [[pps.tile([128, 360], f32, name=f"pp{mc}{nch}", tag=f"pp{mc}{nch}")
                        for nch in range(2)] for mc in range(2)]
                for pos in range(81):
                    ky, kx = pos // 9, pos % 9
                    wt = wpool.tile([128, 512], bf16, name="w", tag="w")
                    (nc.sync if pos % 2 == 0 else nc.scalar).dma_start(
                        out=wt[:],
                        in_=D(primw_d, [(512, 128), (1, 512)], pos * 128 * 512))
                    for kc in range(2):
                        for mc in range(2):
                            lhsT = wt[:, kc * 256 + mc * 128:
                                      kc * 256 + (mc + 1) * 128]
                            for nch in range(2):
                                rhs = A(SPIKES[kc], 0, 128,
                                        [(400, 10), (40, 6), (2, 6)],
                                        nch * 4000 + ky * 20 + kx)
                                nc.tensor.matmul(
                                    out=PSP[mc][nch][:, 0:360],
                                    lhsT=lhsT, rhs=rhs,
                                    start=(pos == 0 and kc == 0),
                                    stop=(pos == 80 and kc == 1))

                # --- prim evac (+bias), membranes, spikes ---
                PRIM = [cpool.tile([128, T * BL * 36], f32, name=f"pr{mc}", tag=f"pr{mc}")
                        for mc in range(2)]
                PSPK = [cpool.tile([128, T * BL * 36], bf16, name=f"ps{mc}", tag=f"ps{mc}")
                        for mc in range(2)]
                MPP = [cpool.tile([128, BL * 36], f32, name=f"mpp{mc}", tag=f"mpp{mc}")
                       for mc in range(2)]
                APP = [cpool.tile([128, BL * 36], f32, name=f"app{mc}", tag=f"app{mc}")
                       for mc in range(2)]
                for mc in range(2):
                    for nch in range(2):
                        nc.vector.tensor_scalar(
                            out=PRIM[mc][:, nch * 360:(nch + 1) * 360],
                            in0=PSP[mc][nch][:, 0:360],
                            scalar1=PB[:, mc:mc + 1], scalar2=None,
                            op0=Alu.add)
                if stage == "prim":
                    nc.sync.dma_start(out=D(outm_d, [(32, 128), (1, 32)]),
                                      in_=A(PRIM[0], 0, 128, [(1, 32)]))
                    return
                for t in range(T):
                    for mc in range(2):
                        po = A(PRIM[mc], 0, 128, [(1, 144)], t * 144)
                        sp = A(PSPK[mc], 0, 128, [(1, 144)], t * 144)
                        if t == 0:
                            nc.vector.tensor_scalar(
                                out=sp, in0=po, scalar1=1.0, scalar2=None,
                                op0=Alu.is_gt)
                            nc.gpsimd.tensor_tensor(
                                out=MPP[mc][:], in0=po, in1=sp,
                                op=Alu.subtract)
                        else:
                            nc.vector.scalar_tensor_tensor(
                                out=APP[mc][:], in0=MPP[mc][:], scalar=0.2,
                                in1=po, op0=Alu.mult, op1=Alu.add)
                            nc.vector.tensor_scalar(
                                out=sp, in0=APP[mc][:], scalar1=1.0,
                                scalar2=None, op0=Alu.is_gt)
                            if t < T - 1:
                                nc.gpsimd.tensor_tensor(
                                    out=MPP[mc][:], in0=APP[mc][:], in1=sp,
                                    op=Alu.subtract)

                # --- prim spikes -> a2a_in ---
                # flat f = (mc*128+p)*36 + pix ; dest block j = f//1152,
                # rem = f%1152. Split partitions in groups of 32 so j is
                # constant per DMA:  p = ph*32 + pl ->
                # dst = j*23040 + b*5760 + t*1152 + pl*36 + pix, j = mc*4+ph
                env_spk0 = SPIKES[0]
                dmae = [nc.sync, nc.scalar, nc.gpsimd]
                di = 0
                for mc in range(2):
                    for ph in range(4):
                        j = mc * 4 + ph
                        for t in range(T):
                            src = A(PSPK[mc], ph * 32, 32,
                                    [(36, BL), (1, 36)], t * 144)
                            dst = D(a2a_in,
                                    [(36, 32), (5760, BL), (1, 36)],
                                    j * 23040 + t * 1152)
                            dmae[di % 3].dma_start(out=dst, in_=src)
                            di += 1

            if stage == "conv":
                nc.gpsimd.dma_start(out=D(outm_d, [(32, 128), (1, 32)]),
                                    in_=A(env_spk0, 0, 128, [(1, 32)]))
                return
            # =========== AllToAll: batch-shard -> route-shard ===========
            if solo:
                nc.sync.dma_start(out=a2a_out[:], in_=a2a_in[:])
            else:
                nc.gpsimd.collective_compute(
                    "AllToAll", Alu.bypass, replica_groups=rg,
                    ins=[a2a_in[:]], outs=[a2a_out[:]])

            if stage == "a2a":
                nc.gpsimd.dma_start(out=D(outm_d, [(32, 128), (1, 32)]),
                                    in_=D(a2a_out, [(32, 128), (1, 32)]))
                return
            # =========== Phase B prep: transposes + u_hat ===========
            # UH [128, T*5760]: col = t*5760 + rslot*32 + b.
            #   rslot 0..144: main co (partitions = co 0..128)
            #   rslot 144..180: tail co, partitions = (rq, cot) fold.
            # Digit bias (2x domain) is folded into the PSUM evac, so the
            # membrane threshold is a uniform 1.0 for all partitions/steps.
            NS = 5760
            uall = stk.enter_context(tc.tile_pool(name="uall", bufs=1))
            UH = uall.tile([128, T * NS], f32, name="uh", tag="uh")
            BIAV = uall.tile([128, 2], f32, name="biav", tag="biav")
            nc.sync.dma_start(out=BIAV[:], in_=biav_d[:])
            with ExitStack() as ustk:
                upool = ustk.enter_context(tc.tile_pool(name="uh", bufs=1))
                w2pool = ustk.enter_context(tc.tile_pool(name="w2c", bufs=3))
                ups = ustk.enter_context(tc.tile_pool(name="upsum", bufs=4,
                                                      space="PSUM"))

                IDT = upool.tile([32, 32], bf16, name="idt", tag="idt")
                from concourse.masks import make_identity
                make_identity(nc, IDT[:])
                X16 = [persist.tile([128, 160], bf16, name=f"x16_{g}",
                                    tag=f"x16_{g}") for g in range(9)]
                for g in range(9):
                    Mg = upool.tile([32, T * 128], bf16, name="mg", tag="mg",
                                    bufs=2)
                    nc.sync.dma_start(
                        out=Mg[:],
                        in_=D(a2a_out, [(5760, 32), (1152, T), (1, 128)],
                              g * 128))
                    for t in range(T):
                        pst = ups.tile([128, 32], bf16, name="pst", tag="pst", bufs=2)
                        nc.tensor.transpose(
                            out=pst[:], in_=Mg[:, t * 128:(t + 1) * 128],
                            identity=IDT[:])
                        nc.vector.tensor_copy(
                            out=X16[g][:, t * 32:(t + 1) * 32], in_=pst[:])

                if stage == "trans":
                    nc.gpsimd.dma_start(
                        out=D(outm_d, [(32, 128), (1, 32)]),
                        in_=A(X16[0], 0, 128, [(1, 32)]))
                    return
                ActF = mybir.ActivationFunctionType
                for g in range(9):
                    w2c = w2pool.tile([128, 16 * CO], bf16, name="w2c",
                                      tag="w2c", bufs=2)
                    (nc.sync if g % 2 == 0 else nc.scalar).dma_start(
                        out=w2c[:],
                        in_=D(w2g_d, [(2560, 128), (1, 2560)],
                              g * 128 * 2560))
                    for r2 in range(8):
                        psA = ups.tile([128, 320], f32, name="upa", tag="upa",
                                       bufs=3)
                        psB = ups.tile([128, 320], f32, name="upb", tag="upb",
                                       bufs=3)
                        for j in range(2):
                            rr = r2 * 2 + j
                            r = g * 16 + rr
                            rq = r // 36
                            q = (rr // 4) * 32
                            rhs = A(X16[g], q, 32, [(1, 160)])
                            nc.tensor.matmul(
                                out=psA[:, j * 160:(j + 1) * 160],
                                lhsT=A(w2c, q, 32, [(1, 128)], rr * CO),
                                rhs=rhs, start=True, stop=True,
                                tile_position=(q, 0))
                            nc.tensor.matmul(
                                out=A(psB, rq * 32, 32, [(1, 160)], j * 160),
                                lhsT=A(w2c, q, 32, [(1, 32)], rr * CO + 128),
                                rhs=rhs, start=True, stop=True,
                                tile_position=(q, rq * 32))
                        r0 = g * 16 + r2 * 2
                        rq0, rl0 = r0 // 36, r0 % 36
                        # evac + per-partition bias add (Act for main on
                        # 128 parts, DVE for the tail band).
                        nc.scalar.activation(
                            out=A(UH, 0, 128, [(32, 2), (NS, T), (1, 32)],
                                  r0 * 32),
                            in_=A(psA, 0, 128, [(160, 2), (32, T), (1, 32)]),
                            func=ActF.Identity,
                            bias=BIAV[:, 0:1], scale=1.0)
                        nc.vector.tensor_scalar(
                            out=A(UH, rq0 * 32, 32,
                                  [(32, 2), (NS, T), (1, 32)],
                                  (144 + rl0) * 32),
                            in0=A(psB, rq0 * 32, 32,
                                  [(160, 2), (32, T), (1, 32)]),
                            scalar1=A(BIAV, rq0 * 32, 32, [(1, 1)], 1),
                            scalar2=None, op0=Alu.add)

            if stage == "uhat":
                nc.gpsimd.dma_start(out=D(outm_d, [(32, 128), (1, 32)]),
                                    in_=A(X16[0], 0, 128, [(1, 32)]))
                return
            # =========== Phase B: digit-caps loop (route-sharded) ========
            dpool = stk.enter_context(tc.tile_pool(name="dig", bufs=1))
            dps = stk.enter_context(tc.tile_pool(name="dpsum", bufs=1,
                                                 space="PSUM"))

            SELAb = dpool.tile([128, 160], bf16, name="sela", tag="sela")
            SELTb = dpool.tile([32, 160], bf16, name="selt", tag="selt")
            REPL = dpool.tile([32, 128], bf16, name="repl", tag="repl")
            FOLD = dpool.tile([128, 32], f32, name="fold", tag="fold")
            nc.sync.dma_start(out=SELAb[:], in_=sela_d[:])
            nc.sync.dma_start(out=SELTb[:], in_=selt_d[:])
            nc.sync.dma_start(out=REPL[:], in_=repl_d[:])
            nc.sync.dma_start(out=FOLD[:], in_=fold_d[:])
            IDT128 = dpool.tile([128, 128], f32, name="idt128", tag="idt128")
            from concourse.masks import make_identity as _mkid
            _mkid(nc, IDT128[:])

            DSB = [dpool.tile([128, NS], bf16, name=f"dsb{i}", tag=f"dsb{i}")
                   for i in range(2)]
            TR = dpool.tile([128, NS], bf16, name="tr", tag="tr")
            Y = dpool.tile([128, NS], bf16, name="y", tag="y")
            Z = dpool.tile([128, NS], bf16, name="z", tag="z")
            BIJf = dpool.tile([128, 180], f32, name="bijf", tag="bijf")
            BIJb = dpool.tile([128, 180], bf16, name="bijb", tag="bijb")
            SJF = dpool.tile([128, 32], f32, name="sjf", tag="sjf")
            SJQ = dpool.tile([128, 32], f32, name="sjq", tag="sjq")
            SJG_m = dpool.tile([128, 32], f32, name="sjgm", tag="sjgm")
            SJG_t = dpool.tile([32, 32], f32, name="sjgt", tag="sjgt")
            SJT = dpool.tile([32, 32], f32, name="sjt", tag="sjt")
            SJGA = dpool.tile([128, 256], f32, name="sjga", tag="sjga")
            SJGB = dpool.tile([32, 256], f32, name="sjgb", tag="sjgb")
            PDTS = dpool.tile([32, 145], f32, name="pdts", tag="pdts")
            OUT_m = dpool.tile([128, 32], f32, name="outm", tag="outm")
            OUT_t = dpool.tile([32, 32], f32, name="outt", tag="outt")
            A2_m = dpool.tile([128, 32], f32, name="a2m", tag="a2m")
            A2_t = dpool.tile([32, 32], f32, name="a2t", tag="a2t")
            M2_m = dpool.tile([128, 32], f32, name="m2m", tag="m2m")
            M2_t = dpool.tile([32, 32], f32, name="m2t", tag="m2t")
            D2m = dpool.tile([128, 32], bf16, name="d2m", tag="d2m")
            D2t = dpool.tile([32, 32], bf16, name="d2t", tag="d2t")
            D2F = dpool.tile([128, 32], bf16, name="d2f", tag="d2f")
            SCR_m = dpool.tile([128, 32], f32, name="scrm", tag="scrm")
            SCR_t = dpool.tile([32, 32], f32, name="scrt", tag="scrt")
            DSU_m = dpool.tile([128, 1], f32, name="dsum", tag="dsum")
            DSU_t = dpool.tile([32, 1], f32, name="dsut", tag="dsut")
            DSUbm = dpool.tile([128, 1], bf16, name="dsubm", tag="dsubm")
            DSUbt = dpool.tile([32, 1], bf16, name="dsubt", tag="dsubt")
            ZBt = dpool.tile([128, 36], f32, name="zbt", tag="zbt")
            ZBT4 = dpool.tile([32, 144], bf16, name="zbt4", tag="zbt4")
            DPDF = dpool.tile([128, 37], f32, name="dpdf", tag="dpdf")
            PDSm = dpool.tile([128, 1], f32, name="pdsm", tag="pdsm")

            bij0 = float(np.float32(1.0) / np.float32(R))
            nc.vector.memset(BIJf[:], bij0)
            nc.vector.tensor_copy(out=BIJb[:], in_=BIJf[:])

            # AP views
            def Uf(t):
                return A(UH, 0, 128, [(1, NS)], t * NS)

            def Ur(t, s0=0, cnt=180):
                return A(UH, 0, 128, [(32, cnt), (1, 32)], t * NS + s0 * 32)

            def DSr(i):   # (rslot, b) iteration over (b,r)-major storage
                return A(DSB[i], 0, 128, [(1, 180), (180, 32)])

            def DSbb(i):  # (b, rslot) iteration
                return A(DSB[i], 0, 128, [(180, 32), (1, 180)])

            TRr = A(TR, 0, 128, [(32, 180), (1, 32)])

            # prologue: spikes at t=0 (m0 = u0), trace init = ds0
            nc.vector.tensor_scalar(out=DSr(0), in0=Ur(0), scalar1=1.0,
                                    scalar2=None, op0=Alu.is_gt)
            nc.gpsimd.tensor_tensor(out=Ur(0), in0=Ur(0), in1=DSr(0),
                                    op=Alu.subtract)
            nc.scalar.copy(out=TR[:], in_=DSB[0][:])

            for t in range(T):
                i = t % 2
                # ---- y = ds * bij ; s_j partials; local OUT accum ----
                nc.vector.tensor_tensor(
                    out=A(Y, 0, 128, [(180, 32), (1, 180)]),
                    in0=DSbb(i),
                    in1=A(BIJb, 0, 128, [(0, 32), (1, 180)]),
                    op=Alu.mult)
                nc.vector.tensor_reduce(
                    out=SJF[:], in_=A(Y, 0, 128, [(180, 32), (1, 144)]),
                    axis=mybir.AxisListType.X, op=Alu.add)
                nc.vector.tensor_reduce(
                    out=SJQ[:], in_=A(Y, 0, 128, [(180, 32), (1, 36)], 144),
                    axis=mybir.AxisListType.X, op=Alu.add)
                SJTps = dps.tile([32, 32], f32, name="sjtp", tag="sjtp")
                nc.tensor.matmul(out=SJTps[:], lhsT=FOLD[:], rhs=SJQ[:],
                                 start=True, stop=True)
                nc.vector.tensor_copy(out=SJT[:], in_=SJTps[:])
                if t == 0:
                    nc.vector.tensor_copy(out=OUT_m[:], in_=SJF[:])
                    nc.vector.tensor_copy(out=OUT_t[:], in_=SJT[:])
                else:
                    nc.vector.tensor_tensor(out=OUT_m[:], in0=OUT_m[:],
                                            in1=SJF[:], op=Alu.add)
                    nc.vector.tensor_tensor(out=OUT_t[:], in0=OUT_t[:],
                                            in1=SJT[:], op=Alu.add)
                if t == T - 1:
                    break
                # ---- exchange s_j partials (AllReduce) ----
                nc.sync.dma_start(out=D(sj_in[t], [(32, 128), (1, 32)]),
                                  in_=SJF[:])
                nc.sync.dma_start(out=D(sj_in[t], [(32, 32), (1, 32)], 4096),
                                  in_=SJT[:])
                if solo:
                    for _k in range(N_CORES):
                        nc.sync.dma_start(
                            out=D(sj_out[t], [(1, SJP)], _k * SJP),
                            in_=sj_in[t][:])
                else:
                    nc.gpsimd.collective_compute(
                        "AllGather", Alu.bypass, replica_groups=rg,
                        ins=[sj_in[t][:]], outs=[sj_out[t][:]])

                # ---- membranes for t+1 (overlaps the collective) ----
                i2 = (t + 1) % 2
                nc.vector.scalar_tensor_tensor(
                    out=Uf(t + 1), in0=Uf(t), scalar=0.2, in1=Uf(t + 1),
                    op0=Alu.mult, op1=Alu.add)
                nc.vector.tensor_scalar(out=DSr(i2), in0=Ur(t + 1),
                                        scalar1=1.0, scalar2=None,
                                        op0=Alu.is_gt)
                if t + 1 < T - 1:
                    nc.gpsimd.tensor_tensor(out=Ur(t + 1), in0=Ur(t + 1),
                                            in1=DSr(i2), op=Alu.subtract)

                # ---- consume AllReduce t: dig2 chain ----
                nc.sync.dma_start(
                    out=A(SJGA, 0, 128, [(32, 8), (1, 32)]),
                    in_=D(sj_out[t], [(32, 128), (SJP, 8), (1, 32)]))
                nc.sync.dma_start(
                    out=A(SJGB, 0, 32, [(32, 8), (1, 32)]),
                    in_=D(sj_out[t], [(32, 32), (SJP, 8), (1, 32)], 4096))
                nc.vector.tensor_reduce(
                    out=SJG_m[:], in_=A(SJGA, 0, 128, [(1, 32), (32, 8)]),
                    axis=mybir.AxisListType.X, op=Alu.add)
                nc.vector.tensor_reduce(
                    out=SJG_t[:], in_=A(SJGB, 0, 32, [(1, 32), (32, 8)]),
                    axis=mybir.AxisListType.X, op=Alu.add)
                if t == 0:
                    a2m, a2t = SJG_m, SJG_t
                else:
                    nc.vector.scalar_tensor_tensor(
                        out=A2_m[:], in0=M2_m[:], scalar=0.2, in1=SJG_m[:],
                        op0=Alu.mult, op1=Alu.add)
                    nc.vector.scalar_tensor_tensor(
                        out=A2_t[:], in0=M2_t[:], scalar=0.2, in1=SJG_t[:],
                        op0=Alu.mult, op1=Alu.add)
                    a2m, a2t = A2_m, A2_t
                nc.vector.tensor_scalar(out=D2m[:], in0=a2m[:], scalar1=0.5,
                                        scalar2=None, op0=Alu.is_gt)
                nc.vector.tensor_scalar(out=D2t[:], in0=a2t[:], scalar1=0.5,
                                        scalar2=None, op0=Alu.is_gt)
                if t < T - 2:
                    nc.vector.scalar_tensor_tensor(
                        out=M2_m[:], in0=D2m[:], scalar=-0.5, in1=a2m[:],
                        op0=Alu.mult, op1=Alu.add)
                    nc.vector.scalar_tensor_tensor(
                        out=M2_t[:], in0=D2t[:], scalar=-0.5, in1=a2t[:],
                        op0=Alu.mult, op1=Alu.add)
                # replicate tail d2s to the (rq,cot) fold layout via PE
                REPps = dps.tile([128, 32], f32, name="repp", tag="repp")
                nc.tensor.matmul(out=REPps[:], lhsT=REPL[:], rhs=D2t[:],
                                 start=True, stop=True)
                nc.vector.tensor_copy(out=D2F[:], in_=REPps[:])
                # -0.1 * alpha * sum_b d2s terms
                nc.vector.tensor_scalar(
                    out=SCR_m[:], in0=D2m[:],
                    scalar1=float(np.float32(0.1) * ALPHA), scalar2=None,
                    op0=Alu.mult, op1=Alu.add, accum_out=DSU_m[:])
                nc.vector.tensor_scalar(
                    out=SCR_t[:], in0=D2t[:],
                    scalar1=float(np.float32(0.1) * ALPHA), scalar2=None,
                    op0=Alu.mult, op1=Alu.add, accum_out=DSU_t[:])
                nc.vector.tensor_copy(out=DSUbm[:], in_=DSU_m[:])
                nc.vector.tensor_copy(out=DSUbt[:], in_=DSU_t[:])

                # ---- z = trace * d2s ; tail zb ----
                nc.vector.tensor_tensor(
                    out=A(Z, 0, 128, [(32, 144), (1, 32)]),
                    in0=A(TR, 0, 128, [(32, 144), (1, 32)]),
                    in1=A(D2m, 0, 128, [(0, 144), (1, 32)]),
                    op=Alu.mult)
                nc.vector.tensor_tensor(
                    out=A(Z, 0, 128, [(32, 36), (1, 32)], 144 * 32),
                    in0=A(TR, 0, 128, [(32, 36), (1, 32)], 144 * 32),
                    in1=A(D2F, 0, 128, [(0, 36), (1, 32)]),
                    op=Alu.mult)
                nc.vector.tensor_reduce(
                    out=ZBt[:, 0:36],
                    in_=A(Z, 0, 128, [(32, 36), (1, 32)], 144 * 32),
                    axis=mybir.AxisListType.X, op=Alu.add)
                # fold ZBt [128,36] -> ZBT4 [32, (rq,36)] via PE
                ZBT4ps = dps.tile([32, 144], f32, name="zbt4p", tag="zbt4p")
                for rq in range(4):
                    nc.tensor.matmul(
                        out=A(ZBT4ps, 0, 32, [(1, 36)], rq * 36),
                        lhsT=IDT128[:, rq * 32:(rq + 1) * 32],
                        rhs=ZBt[:, 0:36],
                        start=(rq == 0), stop=(rq == 3))
                nc.vector.tensor_copy(out=ZBT4[:], in_=ZBT4ps[:])

                # ---- delta matmuls: PE accumulates over b ----
                PD_m = dps.tile([128, 144], f32, name="pdm", tag="pdm")
                PD_t = dps.tile([32, 144], f32, name="pdt", tag="pdt")
                PDm1 = dps.tile([128, 1], f32, name="pdm1", tag="pdm1")
                PDt1 = dps.tile([32, 1], f32, name="pdt1", tag="pdt1")
                for b in range(32):
                    nc.tensor.matmul(out=PD_m[:, 0:144],
                                     lhsT=SELAb[:, 0:128],
                                     rhs=A(Z, 0, 128, [(32, 144)], b),
                                     start=(b == 0), stop=False)
                for b in range(32):
                    nc.tensor.matmul(out=PD_t[:, 0:144],
                                     lhsT=SELAb[:, 128:160],
                                     rhs=A(Z, 0, 128, [(32, 144)], b),
                                     start=(b == 0), stop=False)
                nc.tensor.matmul(out=PD_m[:, 0:144], lhsT=SELTb[:, 0:128],
                                 rhs=ZBT4[:], start=False, stop=True)
                nc.tensor.matmul(out=PD_t[:, 0:144], lhsT=SELTb[:, 128:160],
                                 rhs=ZBT4[:], start=False, stop=True)
                nc.tensor.matmul(out=PDm1[:], lhsT=SELAb[:, 0:128],
                                 rhs=DSUbm[:], start=True, stop=False)
                nc.tensor.matmul(out=PDm1[:], lhsT=SELTb[:, 0:128],
                                 rhs=DSUbt[:], start=False, stop=True)
                nc.tensor.matmul(out=PDt1[:], lhsT=SELAb[:, 128:160],
                                 rhs=DSUbm[:], start=True, stop=False)
                nc.tensor.matmul(out=PDt1[:], lhsT=SELTb[:, 128:160],
                                 rhs=DSUbt[:], start=False, stop=True)

                # ---- bij updates ----
                nc.vector.scalar_tensor_tensor(
                    out=A(BIJf, 0, 128, [(1, 144)]),
                    in0=PD_m[:, 0:144], scalar=float(ALPHA),
                    in1=A(BIJf, 0, 128, [(1, 144)]),
                    op0=Alu.mult, op1=Alu.add)
                nc.vector.tensor_copy(out=PDSm[:], in_=PDm1[:])
                nc.vector.tensor_scalar(
                    out=A(BIJf, 0, 128, [(1, 144)]),
                    in0=A(BIJf, 0, 128, [(1, 144)]),
                    scalar1=PDSm[:, 0:1], scalar2=None, op0=Alu.subtract)
                # tail: partition-expand PD_t/PDt1 via small DMAs
                nc.vector.tensor_copy(out=PDTS[:, 0:144], in_=PD_t[:])
                nc.vector.tensor_copy(out=PDTS[:, 144:145], in_=PDt1[:])
                dmaq = [nc.sync, nc.scalar]
                for rq in range(4):
                    dmaq[rq % 2].dma_start(
                        out=A(DPDF, rq * 32, 32, [(1, 36)]),
                        in_=A(PDTS, 0, 32, [(1, 36)], rq * 36))
                    dmaq[rq % 2].dma_start(
                        out=A(DPDF, rq * 32, 32, [(1, 1)], 36),
                        in_=PDTS[:, 144:145])
                nc.vector.scalar_tensor_tensor(
                    out=A(BIJf, 0, 128, [(1, 36)], 144),
                    in0=DPDF[:, 0:36], scalar=float(ALPHA),
                    in1=A(BIJf, 0, 128, [(1, 36)], 144),
                    op0=Alu.mult, op1=Alu.add)
                nc.vector.tensor_scalar(
                    out=A(BIJf, 0, 128, [(1, 36)], 144),
                    in0=A(BIJf, 0, 128, [(1, 36)], 144),
                    scalar1=DPDF[:, 36:37], scalar2=None, op0=Alu.subtract)
                nc.vector.tensor_copy(out=BIJb[:], in_=BIJf[:])

                # ---- trace update for t+1 (after z(t) consumed TR) ----
                # Pool rejects scalar_tensor_tensor on TRN2; split into a
                # DVE 4x tensor_scalar (into Z scratch) + Pool max.
                if t < T - 2:
                    nc.vector.tensor_scalar(
                        out=Z[:], in0=TR[:], scalar1=float(DECAY_TR),
                        scalar2=None, op0=Alu.mult)
                    nc.vector.tensor_tensor(
                        out=A(TR, 0, 128, [(32, 180), (1, 32)]),
                        in0=A(Z, 0, 128, [(32, 180), (1, 32)]),
                        in1=DSr(i2), op=Alu.max)

            # ---- write outputs (local partial sums; host adds cores) ----
            nc.sync.dma_start(out=D(outm_d, [(32, 128), (1, 32)]),
                              in_=OUT_m[:])
            nc.sync.dma_start(out=D(outm_d, [(32, 32), (1, 32)], 128 * 32),
                              in_=OUT_t[:])
            nc.sync.dma_start(out=dbg_d[:], in_=A(UH, 0, 128, [(1, 512)]))


def _host_prepare(data, conv_w, conv_b, prim_w, prim_b, W, bias):
    """Build per-core input maps."""
    from numpy.lib.stride_tricks import sliding_window_view
    f32 = np.float32
    data = np.asarray(data, f32)
    conv_w = np.asarray(conv_w, f32)
    conv_b = np.asarray(conv_b, f32)
    prim_w = np.asarray(prim_w, f32)
    prim_b = np.asarray(prim_b, f32)
    W = np.asarray(W, f32)
    bias = np.asarray(bias, f32)

    # im2col: win[b, ky, kx, oy, ox]
    win = sliding_window_view(data[:, 0, :, :], (20, 20), axis=(1, 2))
    im2_all = np.ascontiguousarray(win).reshape(B, 81, 400)

    # everything feeding the spiking membranes runs in a 2x-scaled domain
    # (exact in fp32) so the reset is the plain subtract M = A - ds.
    convw = np.ascontiguousarray(conv_w.reshape(256, 81).T) * f32(2.0)
    convb2 = np.ascontiguousarray(conv_b.reshape(2, 128).T) * f32(2.0)

    import ml_dtypes
    bf16 = ml_dtypes.bfloat16
    pw = prim_w.reshape(2, 128, 2, 128, 9, 9)
    primw = (np.ascontiguousarray(
        pw.transpose(4, 5, 3, 2, 0, 1).reshape(81, 128, 512))
        * f32(2.0)).astype(bf16)
    primb2 = np.ascontiguousarray(prim_b.reshape(2, 128).T) * f32(2.0)

    # W2[i, r, co] with co = o*10 + c, zero-padded to K=32 route-quads:
    # w2g[g, rr*8+i, rr*160+co] = 2*W2[i, g*16+rr, co]
    Wt = np.ascontiguousarray(
        W.transpose(3, 0, 2, 1)).reshape(8, R, CO) * f32(2.0)

    # per-partition digit bias (2x domain), col0 = main co, col1 = tail fold
    bias_o = bias[:, 0]
    biav = np.zeros((128, 2), f32)
    for p in range(128):
        biav[p, 0] = f32(2.0) * bias_o[p // 10]
        biav[p, 1] = f32(2.0) * bias_o[(128 + p % 32) // 10]

    cos = np.arange(CO)
    sela = np.ascontiguousarray(
        (np.equal.outer(cos[:128] % 10, cos % 10)).astype(bf16))
    selt = np.ascontiguousarray(
        (np.equal.outer(cos[128:] % 10, cos % 10)).astype(bf16))
    # repl[k, m] = [k == m % 32]  (replicate [32,x] -> [128,x] via PE)
    repl = np.ascontiguousarray(
        np.equal.outer(np.arange(32), np.arange(128) % 32).astype(bf16))
    # fold[p, m] = [p % 32 == m]  (sum 4 rq-groups of partitions)
    fold = np.ascontiguousarray(
        np.equal.outer(np.arange(128) % 32, np.arange(32)).astype(f32))

    in_maps = []
    for k in range(N_CORES):
        im2 = np.ascontiguousarray(
            im2_all[BL * k:BL * (k + 1)].transpose(1, 0, 2).reshape(81, 1600))
        w2core = Wt[:, RL * k:RL * (k + 1), :]          # [8, 144, 160]
        w2g = np.zeros((9, 128, 16 * CO), bf16)
        for rr in range(16):
            # [8, 9, 160] block for this rr across all 9 groups
            blk = w2core[:, rr::16, :]
            w2g[:, rr * 8:(rr + 1) * 8, rr * CO:(rr + 1) * CO] = \
                blk.transpose(1, 0, 2)
        in_maps.append({
            "im2": im2, "convw": convw, "convb": convb2,
            "primw": primw, "primb": primb2, "w2g": w2g,
            "biav": biav, "sela": sela, "selt": selt,
            "repl": repl, "fold": fold,
        })
    return in_maps


HOST_SUM_OUT = True


def _postprocess(outm):
    """outm [160, 32] (co = o*10+c) -> classes [32, 10]."""
    out3 = outm.reshape(16, 10, 32).astype(np.float32) / np.float32(T)
    sq = (out3 * out3).sum(axis=0)
    return np.sqrt(sq).T.astype(np.float32)


def kernel(data, conv_w, conv_b, prim_w, prim_b, W, bias, time_window):
    from concourse.bass_utils import run_bass_kernel_spmd
    assert int(time_window) == T
    if "nc" not in _CACHE:
        _CACHE["nc"] = _build_program()
    nc = _CACHE["nc"]
    in_maps = _host_prepare(data, conv_w, conv_b, prim_w, prim_b, W, bias)
    res = run_bass_kernel_spmd(nc, in_maps, core_ids=list(range(N_CORES)))
    outm = np.sum([np.asarray(res.results[k]["outm"], np.float32)
                   for k in range(N_CORES)], axis=0)
    return _postprocess(outm)



# revision 27
# speedup vs baseline: 1.2375x; 1.2006x over previous
"""Trainium2 Bass kernel for the spiking CapsNet forward pass (nn_CapsNet).

Strategy (8 NeuronCores):
  Phase A (batch-parallel, 4 images/core):
    conv1 once (input is constant over the 5 timesteps), conv-layer membrane
    dynamics for all 5 steps upfront, then the expensive prim conv batched
    over all 5 timesteps in one weight-stationary bf16 PE pass (conv spikes
    are exactly representable in bf16).
  AllToAll (bf16): re-shard prim spikes from batch-split to route-split.
  Phase B (route-parallel, 144 routes/core, full batch):
    u_hat on PE per route (bf16, contraction over i=8) with the digit bias
    folded into the PSUM evacuation so the membrane threshold is a uniform
    1.0; digit-caps membrane / trace / routing chain on DVE+Pool+Act with a
    per-step AllGather of the s_j partials; routing delta via PE
    accumulation over the batch. Output accumulates local partials only;
    the host sums across cores.
Host side: input re-layout (im2col, weight transposes, bf16 casts) and the
  final classes = sqrt(sum_o (out_mem/T)^2) reduction.
"""

import numpy as np

N_CORES = 8
T = 5
B = 32
BL = B // N_CORES          # local batch (4)
R = 1152
RL = R // N_CORES          # local routes (144)
CO = 160                   # (o,c) pairs, ordered co = o*10 + c
DECAY = np.float32(0.2)
THRESH = np.float32(0.5)
DECAY_TR = np.float32(np.exp(np.float32(-1.0 / 1.5)))
ALPHA = np.float32(np.float32(0.0008) / np.float32(32.0))

_CACHE = {}


def _build_program(reps=1, stage="full", solo=False):
    import concourse.bass as bass
    import concourse.mybir as mybir
    import concourse.tile as tile
    from concourse import bacc

    Alu = mybir.AluOpType
    f32 = mybir.dt.float32
    bf16 = mybir.dt.bfloat16

    nc = bacc.Bacc("TRN2", target_bir_lowering=False, debug=False,
                   num_devices=N_CORES)

    # ---- external I/O (per-core values supplied by host) ----
    im2_d = nc.dram_tensor("im2", [81, BL * 400], f32, kind="ExternalInput")
    convw_d = nc.dram_tensor("convw", [81, 256], f32, kind="ExternalInput")
    convb_d = nc.dram_tensor("convb", [128, 2], f32, kind="ExternalInput")
    primw_d = nc.dram_tensor("primw", [81, 128, 512], bf16,
                             kind="ExternalInput")
    primb_d = nc.dram_tensor("primb", [128, 2], f32, kind="ExternalInput")
    w2g_d = nc.dram_tensor("w2g", [9, 128, 16 * CO], bf16,
                           kind="ExternalInput")
    biav_d = nc.dram_tensor("biav", [128, 2], f32, kind="ExternalInput")
    sela_d = nc.dram_tensor("sela", [128, 160], bf16, kind="ExternalInput")
    selt_d = nc.dram_tensor("selt", [32, 160], bf16, kind="ExternalInput")
    repl_d = nc.dram_tensor("repl", [32, 128], bf16, kind="ExternalInput")
    fold_d = nc.dram_tensor("fold", [128, 32], f32, kind="ExternalInput")
    outm_d = nc.dram_tensor("outm", [160, 32], f32, kind="ExternalOutput")
    dbg_d = nc.dram_tensor("dbg", [128, 512], f32, kind="ExternalOutput")

    # ---- internal DRAM ----
    # a2a: per-dest-rank blocks [dest 8][b_l 4][t 5][flat 1152]
    a2a_in = nc.dram_tensor("a2a_in", [N_CORES * BL * T * 1152], bf16)
    a2a_out = nc.dram_tensor("a2a_out", [N_CORES * BL * T * 1152], bf16)
    # per-step s_j exchange: AllGather of 32KB partials + local 8-way sum
    SJP = 8192
    sj_in = [nc.dram_tensor(f"sj_in{t}", [SJP], f32) for t in range(T - 1)]
    sj_out = [nc.dram_tensor(f"sj_out{t}", [N_CORES * SJP], f32,
                             addr_space="Shared") for t in range(T - 1)]

    def A(t, p0, pc, dims, foff=0):
        """Raw AP on tile/tensor t: partitions [p0, p0+pc), free dims list
        [(step, count), ...] in elements, plus extra free offset."""
        b = t if isinstance(t, bass.AP) else t[:]
        pitch = b.ap[0][0]
        return bass.AP(b.tensor, b.offset + p0 * pitch + foff,
                       [[pitch, pc]] + [list(d) for d in dims])

    def D(h, dims, off=0):
        """Raw AP on a DRAM tensor handle (flat element space)."""
        b = h[:]
        return bass.AP(b.tensor, off, [list(d) for d in dims])

    rg = [list(range(N_CORES))]

    with tile.TileContext(nc) as tc:
        from contextlib import ExitStack
        for _rep in range(reps):
            _run_once(nc, tc, bass, mybir, locals(), stage, solo)

    nc.compile()
    return nc


def _run_once(nc, tc, bass, mybir, env, stage="full", solo=False):
    import numpy as np
    from contextlib import ExitStack
    Alu = mybir.AluOpType
    f32 = mybir.dt.float32
    bf16 = mybir.dt.bfloat16
    im2_d = env["im2_d"]; convw_d = env["convw_d"]; convb_d = env["convb_d"]
    primw_d = env["primw_d"]; primb_d = env["primb_d"]
    biav_d = env["biav_d"]; sela_d = env["sela_d"]
    repl_d = env["repl_d"]; fold_d = env["fold_d"]
    selt_d = env["selt_d"]; outm_d = env["outm_d"]; dbg_d = env["dbg_d"]
    a2a_in = env["a2a_in"]; a2a_out = env["a2a_out"]
    w2g_d = env["w2g_d"]
    sj_in = env["sj_in"]; sj_out = env["sj_out"]; SJP = env["SJP"]
    A = env["A"]; D = env["D"]; rg = env["rg"]

    if True:
        with ExitStack() as stk:
            # ---------------- persistent pools ----------------
            persist = stk.enter_context(tc.tile_pool(name="persist", bufs=1))
            dram = stk.enter_context(tc.tile_pool(name="dram", bufs=1,
                                                  space="DRAM"))

            # =========== Phase A: conv stage (batch-sharded) ===========
            with ExitStack() as cstk:
                cpool = cstk.enter_context(tc.tile_pool(name="conv", bufs=1))
                wpool = cstk.enter_context(tc.tile_pool(name="wpos", bufs=4))
                cps = cstk.enter_context(tc.tile_pool(name="cpsum", bufs=4,
                                                      space="PSUM"))
                pps = cstk.enter_context(tc.tile_pool(name="ppsum", bufs=1,
                                                      space="PSUM"))

                IM = cpool.tile([81, BL * 400], f32, name="im", tag="im")
                CW = cpool.tile([81, 256], f32, name="cw", tag="cw")
                CB = cpool.tile([128, 2], f32, name="cb", tag="cb")
                PB = cpool.tile([128, 2], f32, name="pb", tag="pb")
                nc.sync.dma_start(out=IM[:], in_=im2_d[:])
                nc.sync.dma_start(out=CW[:], in_=convw_d[:])
                nc.sync.dma_start(out=CB[:], in_=convb_d[:])
                nc.sync.dma_start(out=PB[:], in_=primb_d[:])

                # SPIKES[kc] holds conv spikes for all (t, b): [128, 8000]
                SPIKES = [cpool.tile([128, T * BL * 400], bf16,
                                     name=f"spk{kc}", tag=f"spk{kc}")
                          for kc in range(2)]
                CONVOUT = [cpool.tile([128, BL * 400], f32, name=f"co{kc}",
                                      tag=f"co{kc}") for kc in range(2)]
                MPC = [cpool.tile([128, BL * 400], f32, name=f"mpc{kc}",
                                  tag=f"mpc{kc}") for kc in range(2)]
                ASC = [cpool.tile([128, BL * 400], f32, name=f"asc{kc}",
                                  tag=f"asc{kc}") for kc in range(2)]

                # --- conv1: out[co, (b,pix)] = sum_k convw[k,co] im2[k,(b,pix)]
                for mc in range(2):
                    for b in range(BL):
                        pc = cps.tile([128, 400], f32, name="cvp", tag="cvp")
                        nc.tensor.matmul(
                            out=pc[:, 0:400],
                            lhsT=CW[:, mc * 128:(mc + 1) * 128],
                            rhs=IM[:, b * 400:(b + 1) * 400],
                            start=True, stop=True)
                        # relu(x + bias): exact via DVE tensor_scalar chain
                        nc.vector.tensor_scalar(
                            out=CONVOUT[mc][:, b * 400:(b + 1) * 400],
                            in0=pc[:, 0:400],
                            scalar1=CB[:, mc:mc + 1], scalar2=0.0,
                            op0=Alu.add, op1=Alu.max)

                # --- conv membrane dynamics for all T steps ---
                for t in range(T):
                    for kc in range(2):
                        spk = A(SPIKES[kc], 0, 128, [(1, BL * 400)],
                                t * BL * 400)
                        if t == 0:
                            nc.vector.tensor_scalar(
                                out=spk, in0=CONVOUT[kc][:],
                                scalar1=1.0, scalar2=None, op0=Alu.is_gt)
                            nc.gpsimd.tensor_tensor(
                                out=MPC[kc][:], in0=CONVOUT[kc][:], in1=spk,
                                op=Alu.subtract)
                        else:
                            nc.vector.scalar_tensor_tensor(
                                out=ASC[kc][:], in0=MPC[kc][:], scalar=0.2,
                                in1=CONVOUT[kc][:],
                                op0=Alu.mult, op1=Alu.add)
                            nc.vector.tensor_scalar(
                                out=spk, in0=ASC[kc][:],
                                scalar1=1.0, scalar2=None, op0=Alu.is_gt)
                            if t < T - 1:
                                nc.gpsimd.tensor_tensor(
                                    out=MPC[kc][:], in0=ASC[kc][:], in1=spk,
                                    op=Alu.subtract)

                if stage == "convmem":
                    nc.gpsimd.dma_start(out=D(outm_d, [(32, 128), (1, 32)]),
                                        in_=A(SPIKES[0], 0, 128, [(1, 32)]))
                    return
                # --- prim conv: batched over all (t, b):
                # out[co, (n=(t,b), oy, ox)] accumulated over (kc, ky, kx)
                PSP = [[pps.tile([128, 360], f32, name=f"pp{mc}{nch}",
                                 tag=f"pp{mc}{nch}")
                        for nch in range(2)] for mc in range(2)]
                for pos in range(81):
                    ky, kx = pos // 9, pos % 9
                    wt = wpool.tile([128, 512], bf16, name="w", tag="w")
                    (nc.sync if pos % 2 == 0 else nc.scalar).dma_start(
                        out=wt[:],
                        in_=D(primw_d, [(512, 128), (1, 512)],
                              pos * 128 * 512))
                    for kc in range(2):
                        for mc in range(2):
                            lhsT = wt[:, kc * 256 + mc * 128:
                                      kc * 256 + (mc + 1) * 128]
                            for nch in range(2):
                                rhs = A(SPIKES[kc], 0, 128,
                                        [(400, 10), (40, 6), (2, 6)],
                                        nch * 4000 + ky * 20 + kx)
                                nc.tensor.matmul(
                                    out=PSP[mc][nch][:, 0:360],
                                    lhsT=lhsT, rhs=rhs,
                                    start=(pos == 0 and kc == 0),
                                    stop=(pos == 80 and kc == 1))

                # --- prim evac (+bias), membranes, spikes ---
                PRIM = [cpool.tile([128, T * BL * 36], f32, name=f"pr{mc}",
                                   tag=f"pr{mc}") for mc in range(2)]
                PSPK = [cpool.tile([128, T * BL * 36], bf16, name=f"ps{mc}",
                                   tag=f"ps{mc}") for mc in range(2)]
                MPP = [cpool.tile([128, BL * 36], f32, name=f"mpp{mc}",
                                  tag=f"mpp{mc}") for mc in range(2)]
                APP = [cpool.tile([128, BL * 36], f32, name=f"app{mc}",
                                  tag=f"app{mc}") for mc in range(2)]
                for mc in range(2):
                    for nch in range(2):
                        nc.vector.tensor_scalar(
                            out=PRIM[mc][:, nch * 360:(nch + 1) * 360],
                            in0=PSP[mc][nch][:, 0:360],
                            scalar1=PB[:, mc:mc + 1], scalar2=None,
                            op0=Alu.add)
                if stage == "prim":
                    nc.sync.dma_start(out=D(outm_d, [(32, 128), (1, 32)]),
                                      in_=A(PRIM[0], 0, 128, [(1, 32)]))
                    return
                for t in range(T):
                    for mc in range(2):
                        po = A(PRIM[mc], 0, 128, [(1, 144)], t * 144)
                        sp = A(PSPK[mc], 0, 128, [(1, 144)], t * 144)
                        if t == 0:
                            nc.vector.tensor_scalar(
                                out=sp, in0=po, scalar1=1.0, scalar2=None,
                                op0=Alu.is_gt)
                            nc.gpsimd.tensor_tensor(
                                out=MPP[mc][:], in0=po, in1=sp,
                                op=Alu.subtract)
                        else:
                            nc.vector.scalar_tensor_tensor(
                                out=APP[mc][:], in0=MPP[mc][:], scalar=0.2,
                                in1=po, op0=Alu.mult, op1=Alu.add)
                            nc.vector.tensor_scalar(
                                out=sp, in0=APP[mc][:], scalar1=1.0,
                                scalar2=None, op0=Alu.is_gt)
                            if t < T - 1:
                                nc.gpsimd.tensor_tensor(
                                    out=MPP[mc][:], in0=APP[mc][:], in1=sp,
                                    op=Alu.subtract)

                # --- prim spikes -> a2a_in ---
                # flat f = (mc*128+p)*36 + pix ; dest block j = f//1152,
                # rem = f%1152. Split partitions in groups of 32 so j is
                # constant per DMA:  p = ph*32 + pl ->
                # dst = j*23040 + b*5760 + t*1152 + pl*36 + pix, j = mc*4+ph
                env_spk0 = SPIKES[0]
                dmae = [nc.sync, nc.scalar, nc.gpsimd]
                di = 0
                for mc in range(2):
                    for ph in range(4):
                        j = mc * 4 + ph
                        for t in range(T):
                            src = A(PSPK[mc], ph * 32, 32,
                                    [(36, BL), (1, 36)], t * 144)
                            dst = D(a2a_in,
                                    [(36, 32), (5760, BL), (1, 36)],
                                    j * 23040 + t * 1152)
                            dmae[di % 3].dma_start(out=dst, in_=src)
                            di += 1

            if stage == "conv":
                nc.gpsimd.dma_start(out=D(outm_d, [(32, 128), (1, 32)]),
                                    in_=A(env_spk0, 0, 128, [(1, 32)]))
                return
            # =========== AllToAll: batch-shard -> route-shard ===========
            if solo:
                nc.sync.dma_start(out=a2a_out[:], in_=a2a_in[:])
            else:
                nc.gpsimd.collective_compute(
                    "AllToAll", Alu.bypass, replica_groups=rg,
                    ins=[a2a_in[:]], outs=[a2a_out[:]])

            if stage == "a2a":
                nc.gpsimd.dma_start(out=D(outm_d, [(32, 128), (1, 32)]),
                                    in_=D(a2a_out, [(32, 128), (1, 32)]))
                return
            # =========== Phase B prep: transposes + u_hat ===========
            # UH [128, T*5760]: col = t*5760 + rslot*32 + b.
            #   rslot 0..144: main co (partitions = co 0..128)
            #   rslot 144..180: tail co, partitions = (rq, cot) fold.
            # Digit bias (2x domain) is folded into the PSUM evac, so the
            # membrane threshold is a uniform 1.0 for all partitions/steps.
            NS = 5760
            uall = stk.enter_context(tc.tile_pool(name="uall", bufs=1))
            UH = uall.tile([128, T * NS], f32, name="uh", tag="uh")
            BIAV = uall.tile([128, 2], f32, name="biav", tag="biav")
            nc.sync.dma_start(out=BIAV[:], in_=biav_d[:])
            with ExitStack() as ustk:
                upool = ustk.enter_context(tc.tile_pool(name="uh", bufs=1))
                w2pool = ustk.enter_context(tc.tile_pool(name="w2c", bufs=3))
                ups = ustk.enter_context(tc.tile_pool(name="upsum", bufs=4,
                                                      space="PSUM"))

                IDT = upool.tile([32, 32], bf16, name="idt", tag="idt")
                from concourse.masks import make_identity
                make_identity(nc, IDT[:])
                X16 = [persist.tile([128, 160], bf16, name=f"x16_{g}",
                                    tag=f"x16_{g}") for g in range(9)]
                for g in range(9):
                    Mg = upool.tile([32, T * 128], bf16, name="mg", tag="mg",
                                    bufs=2)
                    nc.sync.dma_start(
                        out=Mg[:],
                        in_=D(a2a_out, [(5760, 32), (1152, T), (1, 128)],
                              g * 128))
                    for t in range(T):
                        pst = ups.tile([128, 32], bf16, name="pst",
                                       tag="pst", bufs=2)
                        nc.tensor.transpose(
                            out=pst[:], in_=Mg[:, t * 128:(t + 1) * 128],
                            identity=IDT[:])
                        nc.vector.tensor_copy(
                            out=X16[g][:, t * 32:(t + 1) * 32], in_=pst[:])

                if stage == "trans":
                    nc.gpsimd.dma_start(
                        out=D(outm_d, [(32, 128), (1, 32)]),
                        in_=A(X16[0], 0, 128, [(1, 32)]))
                    return
                ActF = mybir.ActivationFunctionType
                for g in range(9):
                    w2c = w2pool.tile([128, 16 * CO], bf16, name="w2c",
                                      tag="w2c", bufs=2)
                    (nc.sync if g % 2 == 0 else nc.scalar).dma_start(
                        out=w2c[:],
                        in_=D(w2g_d, [(2560, 128), (1, 2560)],
                              g * 128 * 2560))
                    for r2 in range(8):
                        psA = ups.tile([128, 320], f32, name="upa",
                                       tag="upa", bufs=3)
                        psB = ups.tile([128, 320], f32, name="upb",
                                       tag="upb", bufs=3)
                        for j in range(2):
                            rr = r2 * 2 + j
                            r = g * 16 + rr
                            rq = r // 36
                            q = (rr // 4) * 32
                            rhs = A(X16[g], q, 32, [(1, 160)])
                            nc.tensor.matmul(
                                out=psA[:, j * 160:(j + 1) * 160],
                                lhsT=A(w2c, q, 32, [(1, 128)], rr * CO),
                                rhs=rhs, start=True, stop=True,
                                tile_position=(q, 0))
                            nc.tensor.matmul(
                                out=A(psB, rq * 32, 32, [(1, 160)], j * 160),
                                lhsT=A(w2c, q, 32, [(1, 32)], rr * CO + 128),
                                rhs=rhs, start=True, stop=True,
                                tile_position=(q, rq * 32))
                        r0 = g * 16 + r2 * 2
                        rq0, rl0 = r0 // 36, r0 % 36
                        # evac + per-partition bias add (Act for the main
                        # 128 parts, DVE for the tail band).
                        nc.scalar.activation(
                            out=A(UH, 0, 128, [(32, 2), (NS, T), (1, 32)],
                                  r0 * 32),
                            in_=A(psA, 0, 128, [(160, 2), (32, T), (1, 32)]),
                            func=ActF.Identity,
                            bias=BIAV[:, 0:1], scale=1.0)
                        nc.vector.tensor_scalar(
                            out=A(UH, rq0 * 32, 32,
                                  [(32, 2), (NS, T), (1, 32)],
                                  (144 + rl0) * 32),
                            in0=A(psB, rq0 * 32, 32,
                                  [(160, 2), (32, T), (1, 32)]),
                            scalar1=A(BIAV, rq0 * 32, 32, [(1, 1)], 1),
                            scalar2=None, op0=Alu.add)

            if stage == "uhat":
                nc.gpsimd.dma_start(out=D(outm_d, [(32, 128), (1, 32)]),
                                    in_=A(X16[0], 0, 128, [(1, 32)]))
                return
            # =========== Phase B: digit-caps loop (route-sharded) ========
            dpool = stk.enter_context(tc.tile_pool(name="dig", bufs=1))
            dps = stk.enter_context(tc.tile_pool(name="dpsum", bufs=1,
                                                 space="PSUM"))

            SELAb = dpool.tile([128, 160], bf16, name="sela", tag="sela")
            SELTb = dpool.tile([32, 160], bf16, name="selt", tag="selt")
            REPL = dpool.tile([32, 128], bf16, name="repl", tag="repl")
            FOLD = dpool.tile([128, 32], f32, name="fold", tag="fold")
            nc.sync.dma_start(out=SELAb[:], in_=sela_d[:])
            nc.sync.dma_start(out=SELTb[:], in_=selt_d[:])
            nc.sync.dma_start(out=REPL[:], in_=repl_d[:])
            nc.sync.dma_start(out=FOLD[:], in_=fold_d[:])
            IDT128 = dpool.tile([128, 128], f32, name="idt128", tag="idt128")
            from concourse.masks import make_identity as _mkid
            _mkid(nc, IDT128[:])

            DSB = [dpool.tile([128, NS], bf16, name=f"dsb{i}", tag=f"dsb{i}")
                   for i in range(2)]
            TR = dpool.tile([128, NS], bf16, name="tr", tag="tr")
            Y = dpool.tile([128, NS], bf16, name="y", tag="y")
            Z = dpool.tile([128, NS], bf16, name="z", tag="z")
            BIJf = dpool.tile([128, 180], f32, name="bijf", tag="bijf")
            BIJb = dpool.tile([128, 180], bf16, name="bijb", tag="bijb")
            SJF = dpool.tile([128, 32], f32, name="sjf", tag="sjf")
            SJQ = dpool.tile([128, 32], f32, name="sjq", tag="sjq")
            SJG_m = dpool.tile([128, 32], f32, name="sjgm", tag="sjgm")
            SJG_t = dpool.tile([32, 32], f32, name="sjgt", tag="sjgt")
            SJGA = dpool.tile([128, 256], f32, name="sjga", tag="sjga")
            SJGB1k = dpool.tile([32, 1024], f32, name="sjgb", tag="sjgb")
            OUT_T128 = dpool.tile([128, 32], f32, name="ot128", tag="ot128")
            OUT_m = dpool.tile([128, 32], f32, name="outm", tag="outm")
            OUT_t = dpool.tile([32, 32], f32, name="outt", tag="outt")
            A2_m = dpool.tile([128, 32], f32, name="a2m", tag="a2m")
            A2_t = dpool.tile([32, 32], f32, name="a2t", tag="a2t")
            M2_m = dpool.tile([128, 32], f32, name="m2m", tag="m2m")
            M2_t = dpool.tile([32, 32], f32, name="m2t", tag="m2t")
            D2m = dpool.tile([128, 32], bf16, name="d2m", tag="d2m")
            D2t = dpool.tile([32, 32], bf16, name="d2t", tag="d2t")
            D2F = dpool.tile([128, 32], bf16, name="d2f", tag="d2f")
            SCR_m = dpool.tile([128, 32], f32, name="scrm", tag="scrm")
            SCR_t = dpool.tile([32, 32], f32, name="scrt", tag="scrt")
            DSU_m = dpool.tile([128, 1], f32, name="dsum", tag="dsum")
            DSU_t = dpool.tile([32, 1], f32, name="dsut", tag="dsut")
            DSUbm = dpool.tile([128, 1], bf16, name="dsubm", tag="dsubm")
            DSUbt = dpool.tile([32, 1], bf16, name="dsubt", tag="dsubt")
            ZBt = dpool.tile([128, 36], f32, name="zbt", tag="zbt")
            ZBT4 = dpool.tile([32, 144], bf16, name="zbt4", tag="zbt4")
            DPDF = dpool.tile([128, 37], f32, name="dpdf", tag="dpdf")
            PDSm = dpool.tile([128, 1], f32, name="pdsm", tag="pdsm")
            PDTS = dpool.tile([32, 145], f32, name="pdts", tag="pdts")

            bij0 = float(np.float32(1.0) / np.float32(R))
            nc.vector.memset(BIJf[:], bij0)
            nc.vector.tensor_copy(out=BIJb[:], in_=BIJf[:])

            # AP views
            def Uf(t):
                return A(UH, 0, 128, [(1, NS)], t * NS)

            def Ur(t, s0=0, cnt=180):
                return A(UH, 0, 128, [(32, cnt), (1, 32)], t * NS + s0 * 32)

            def DSr(i):   # (rslot, b) iteration over (b,r)-major storage
                return A(DSB[i], 0, 128, [(1, 180), (180, 32)])

            # prologue: spikes at t=0 (m0 = u0), trace init = ds0
            nc.vector.tensor_scalar(out=DSr(0), in0=Ur(0), scalar1=1.0,
                                    scalar2=None, op0=Alu.is_gt)
            nc.gpsimd.tensor_tensor(out=Ur(0), in0=Ur(0), in1=DSr(0),
                                    op=Alu.subtract)
            nc.scalar.copy(out=TR[:], in_=DSB[0][:])

            RST_P = 62   # reset split: Pool slots [0,62), DVE [62,180)
            for t in range(T):
                i = t % 2
                # ---- y = ds * bij (split main/tail); s_j partials ----
                nc.vector.tensor_tensor(
                    out=A(Y, 0, 128, [(180, 32), (1, 144)]),
                    in0=A(DSB[i], 0, 128, [(180, 32), (1, 144)]),
                    in1=A(BIJb, 0, 128, [(0, 32), (1, 144)]),
                    op=Alu.mult)
                nc.vector.tensor_reduce(
                    out=SJF[:], in_=A(Y, 0, 128, [(180, 32), (1, 144)]),
                    axis=mybir.AxisListType.X, op=Alu.add)
                nc.vector.tensor_tensor(
                    out=A(Y, 0, 128, [(180, 32), (1, 36)], 144),
                    in0=A(DSB[i], 0, 128, [(180, 32), (1, 36)], 144),
                    in1=A(BIJb, 0, 128, [(0, 32), (1, 36)], 144),
                    op=Alu.mult)
                nc.vector.tensor_reduce(
                    out=SJQ[:], in_=A(Y, 0, 128, [(180, 32), (1, 36)], 144),
                    axis=mybir.AxisListType.X, op=Alu.add)
                if t == 0:
                    nc.vector.tensor_copy(out=OUT_m[:], in_=SJF[:])
                    nc.vector.tensor_copy(out=OUT_T128[:], in_=SJQ[:])
                else:
                    nc.vector.tensor_tensor(out=OUT_m[:], in0=OUT_m[:],
                                            in1=SJF[:], op=Alu.add)
                    nc.vector.tensor_tensor(out=OUT_T128[:], in0=OUT_T128[:],
                                            in1=SJQ[:], op=Alu.add)
                if t == T - 1:
                    break
                # ---- exchange s_j partials (AllGather + local sum) ----
                nc.sync.dma_start(out=D(sj_in[t], [(32, 128), (1, 32)]),
                                  in_=SJF[:])
                nc.scalar.dma_start(
                    out=D(sj_in[t], [(32, 128), (1, 32)], 4096), in_=SJQ[:])
                if solo:
                    for _k in range(N_CORES):
                        nc.sync.dma_start(
                            out=D(sj_out[t], [(1, SJP)], _k * SJP),
                            in_=sj_in[t][:])
                else:
                    nc.gpsimd.collective_compute(
                        "AllGather", Alu.bypass, replica_groups=rg,
                        ins=[sj_in[t][:]], outs=[sj_out[t][:]])

                # ---- membranes for t+1 (overlap the collective) ----
                i2 = (t + 1) % 2
                nc.vector.scalar_tensor_tensor(
                    out=Uf(t + 1), in0=Uf(t), scalar=0.2, in1=Uf(t + 1),
                    op0=Alu.mult, op1=Alu.add)
                nc.vector.tensor_scalar(out=DSr(i2), in0=Ur(t + 1),
                                        scalar1=1.0, scalar2=None,
                                        op0=Alu.is_gt)
                if t + 1 < T - 1:
                    # membrane reset, split Pool/DVE to shorten the chain
                    nc.gpsimd.tensor_tensor(
                        out=Ur(t + 1, 0, RST_P), in0=Ur(t + 1, 0, RST_P),
                        in1=A(DSB[i2], 0, 128, [(1, RST_P), (180, 32)]),
                        op=Alu.subtract)
                    nc.vector.tensor_tensor(
                        out=Ur(t + 1, RST_P, 180 - RST_P),
                        in0=Ur(t + 1, RST_P, 180 - RST_P),
                        in1=A(DSB[i2], 0, 128,
                              [(1, 180 - RST_P), (180, 32)], RST_P),
                        op=Alu.subtract)

                # ---- consume AllGather t: dig2 chain ----
                nc.sync.dma_start(
                    out=A(SJGA, 0, 128, [(32, 8), (1, 32)]),
                    in_=D(sj_out[t], [(32, 128), (SJP, 8), (1, 32)]))
                for rq in range(4):
                    (nc.scalar if rq % 2 == 0 else nc.gpsimd).dma_start(
                        out=A(SJGB1k, 0, 32, [(32, 8), (1, 32)], rq * 256),
                        in_=D(sj_out[t], [(32, 32), (SJP, 8), (1, 32)],
                              4096 + rq * 1024))
                nc.vector.tensor_reduce(
                    out=SJG_m[:], in_=A(SJGA, 0, 128, [(1, 32), (32, 8)]),
                    axis=mybir.AxisListType.X, op=Alu.add)
                nc.vector.tensor_reduce(
                    out=SJG_t[:], in_=A(SJGB1k, 0, 32, [(1, 32), (32, 32)]),
                    axis=mybir.AxisListType.X, op=Alu.add)
                if t == 0:
                    a2m, a2t = SJG_m, SJG_t
                else:
                    nc.vector.scalar_tensor_tensor(
                        out=A2_m[:], in0=M2_m[:], scalar=0.2, in1=SJG_m[:],
                        op0=Alu.mult, op1=Alu.add)
                    nc.vector.scalar_tensor_tensor(
                        out=A2_t[:], in0=M2_t[:], scalar=0.2, in1=SJG_t[:],
                        op0=Alu.mult, op1=Alu.add)
                    a2m, a2t = A2_m, A2_t
                nc.vector.tensor_scalar(out=D2m[:], in0=a2m[:], scalar1=0.5,
                                        scalar2=None, op0=Alu.is_gt)
                nc.vector.tensor_scalar(out=D2t[:], in0=a2t[:], scalar1=0.5,
                                        scalar2=None, op0=Alu.is_gt)
                # z-main immediately (does not need the tail replicate)
                nc.vector.tensor_tensor(
                    out=A(Z, 0, 128, [(32, 144), (1, 32)]),
                    in0=A(TR, 0, 128, [(32, 144), (1, 32)]),
                    in1=A(D2m, 0, 128, [(0, 144), (1, 32)]),
                    op=Alu.mult)
                # PE: replicate tail d2s; PD main accumulation over b
                REPps = dps.tile([128, 32], f32, name="repp", tag="repp")
                nc.tensor.matmul(out=REPps[:], lhsT=REPL[:], rhs=D2t[:],
                                 start=True, stop=True)
                PD_m = dps.tile([128, 144], f32, name="pdm", tag="pdm")
                PD_t = dps.tile([32, 144], f32, name="pdt", tag="pdt")
                PDm1 = dps.tile([128, 1], f32, name="pdm1", tag="pdm1")
                PDt1 = dps.tile([32, 1], f32, name="pdt1", tag="pdt1")
                for b in range(32):
                    nc.tensor.matmul(out=PD_m[:, 0:144],
                                     lhsT=SELAb[:, 0:128],
                                     rhs=A(Z, 0, 128, [(32, 144)], b),
                                     start=(b == 0), stop=False)
                for b in range(32):
                    nc.tensor.matmul(out=PD_t[:, 0:144],
                                     lhsT=SELAb[:, 128:160],
                                     rhs=A(Z, 0, 128, [(32, 144)], b),
                                     start=(b == 0), stop=False)
                # Act: evacs off the DVE queue
                nc.scalar.copy(out=D2F[:], in_=REPps[:])
                if t < T - 2:
                    nc.vector.scalar_tensor_tensor(
                        out=M2_m[:], in0=D2m[:], scalar=-0.5, in1=a2m[:],
                        op0=Alu.mult, op1=Alu.add)
                    nc.vector.scalar_tensor_tensor(
                        out=M2_t[:], in0=D2t[:], scalar=-0.5, in1=a2t[:],
                        op0=Alu.mult, op1=Alu.add)
                nc.vector.tensor_scalar(
                    out=SCR_m[:], in0=D2m[:],
                    scalar1=float(np.float32(0.1) * ALPHA), scalar2=None,
                    op0=Alu.mult, op1=Alu.add, accum_out=DSU_m[:])
                nc.vector.tensor_scalar(
                    out=SCR_t[:], in0=D2t[:],
                    scalar1=float(np.float32(0.1) * ALPHA), scalar2=None,
                    op0=Alu.mult, op1=Alu.add, accum_out=DSU_t[:])
                nc.scalar.copy(out=DSUbm[:], in_=DSU_m[:])
                nc.scalar.copy(out=DSUbt[:], in_=DSU_t[:])

                # ---- z tail + zb tail ----
                nc.vector.tensor_tensor(
                    out=A(Z, 0, 128, [(32, 36), (1, 32)], 144 * 32),
                    in0=A(TR, 0, 128, [(32, 36), (1, 32)], 144 * 32),
                    in1=A(D2F, 0, 128, [(0, 36), (1, 32)]),
                    op=Alu.mult)
                nc.vector.tensor_reduce(
                    out=ZBt[:, 0:36],
                    in_=A(Z, 0, 128, [(32, 36), (1, 32)], 144 * 32),
                    axis=mybir.AxisListType.X, op=Alu.add)
                ZBT4ps = dps.tile([32, 144], f32, name="zbt4p", tag="zbt4p")
                for rq in range(4):
                    nc.tensor.matmul(
                        out=A(ZBT4ps, 0, 32, [(1, 36)], rq * 36),
                        lhsT=IDT128[:, rq * 32:(rq + 1) * 32],
                        rhs=ZBt[:, 0:36],
                        start=(rq == 0), stop=(rq == 3))
                nc.vector.tensor_copy(out=ZBT4[:], in_=ZBT4ps[:])
                nc.tensor.matmul(out=PD_m[:, 0:144], lhsT=SELTb[:, 0:128],
                                 rhs=ZBT4[:], start=False, stop=True)
                nc.tensor.matmul(out=PD_t[:, 0:144], lhsT=SELTb[:, 128:160],
                                 rhs=ZBT4[:], start=False, stop=True)
                nc.tensor.matmul(out=PDm1[:], lhsT=SELAb[:, 0:128],
                                 rhs=DSUbm[:], start=True, stop=False)
                nc.tensor.matmul(out=PDm1[:], lhsT=SELTb[:, 0:128],
                                 rhs=DSUbt[:], start=False, stop=True)
                nc.tensor.matmul(out=PDt1[:], lhsT=SELAb[:, 128:160],
                                 rhs=DSUbm[:], start=True, stop=False)
                nc.tensor.matmul(out=PDt1[:], lhsT=SELTb[:, 128:160],
                                 rhs=DSUbt[:], start=False, stop=True)

                # ---- bij main update (fast path for y-main of t+1) ----
                nc.vector.scalar_tensor_tensor(
                    out=A(BIJf, 0, 128, [(1, 144)]),
                    in0=PD_m[:, 0:144], scalar=float(ALPHA),
                    in1=A(BIJf, 0, 128, [(1, 144)]),
                    op0=Alu.mult, op1=Alu.add)
                nc.scalar.copy(out=PDSm[:], in_=PDm1[:])
                nc.vector.tensor_scalar(
                    out=A(BIJf, 0, 128, [(1, 144)]),
                    in0=A(BIJf, 0, 128, [(1, 144)]),
                    scalar1=PDSm[:, 0:1], scalar2=None, op0=Alu.subtract)
                nc.vector.tensor_copy(out=BIJb[:, 0:144],
                                      in_=BIJf[:, 0:144])
                # ---- bij tail: partition-expand via small DMAs ----
                nc.scalar.copy(out=PDTS[:, 0:144], in_=PD_t[:])
                nc.scalar.copy(out=PDTS[:, 144:145], in_=PDt1[:])
                nc.vector.tensor_scalar(
                    out=PDTS[:, 0:144], in0=PDTS[:, 0:144],
                    scalar1=PDTS[:, 144:145], scalar2=None, op0=Alu.subtract)
                dmaq = [nc.sync, nc.scalar, nc.gpsimd, nc.sync]
                for rq in range(4):
                    dmaq[rq].dma_start(
                        out=A(DPDF, rq * 32, 32, [(1, 36)]),
                        in_=A(PDTS, 0, 32, [(1, 36)], rq * 36))
                nc.vector.scalar_tensor_tensor(
                    out=A(BIJf, 0, 128, [(1, 36)], 144),
                    in0=DPDF[:, 0:36], scalar=float(ALPHA),
                    in1=A(BIJf, 0, 128, [(1, 36)], 144),
                    op0=Alu.mult, op1=Alu.add)
                nc.vector.tensor_copy(out=BIJb[:, 144:180],
                                      in_=BIJf[:, 144:180])

                # ---- trace update for t+1 (after z(t) consumed TR) ----
                if t < T - 2:
                    nc.vector.tensor_scalar(
                        out=Z[:], in0=TR[:], scalar1=float(DECAY_TR),
                        scalar2=None, op0=Alu.mult)
                    nc.vector.tensor_tensor(
                        out=A(TR, 0, 128, [(32, 180), (1, 32)]),
                        in0=A(Z, 0, 128, [(32, 180), (1, 32)]),
                        in1=DSr(i2), op=Alu.max)

            # fold the [128,32] tail OUT partials down to [32,32] once
            OTps = dps.tile([32, 32], f32, name="otp", tag="otp")
            nc.tensor.matmul(out=OTps[:], lhsT=FOLD[:], rhs=OUT_T128[:],
                             start=True, stop=True)
            nc.vector.tensor_copy(out=OUT_t[:], in_=OTps[:])
            # ---- write outputs (local partial sums; host adds cores) ----
            nc.sync.dma_start(out=D(outm_d, [(32, 128), (1, 32)]),
                              in_=OUT_m[:])
            nc.sync.dma_start(out=D(outm_d, [(32, 32), (1, 32)], 128 * 32),
                              in_=OUT_t[:])
            nc.sync.dma_start(out=dbg_d[:], in_=A(UH, 0, 128, [(1, 512)]))


def _host_prepare(data, conv_w, conv_b, prim_w, prim_b, W, bias):
    """Build per-core input maps."""
    from numpy.lib.stride_tricks import sliding_window_view
    f32 = np.float32
    data = np.asarray(data, f32)
    conv_w = np.asarray(conv_w, f32)
    conv_b = np.asarray(conv_b, f32)
    prim_w = np.asarray(prim_w, f32)
    prim_b = np.asarray(prim_b, f32)
    W = np.asarray(W, f32)
    bias = np.asarray(bias, f32)

    # im2col: win[b, ky, kx, oy, ox]
    win = sliding_window_view(data[:, 0, :, :], (20, 20), axis=(1, 2))
    im2_all = np.ascontiguousarray(win).reshape(B, 81, 400)

    # everything feeding the spiking membranes runs in a 2x-scaled domain
    # (exact in fp32) so the reset is the plain subtract M = A - ds.
    convw = np.ascontiguousarray(conv_w.reshape(256, 81).T) * f32(2.0)
    convb2 = np.ascontiguousarray(conv_b.reshape(2, 128).T) * f32(2.0)

    import ml_dtypes
    bf16 = ml_dtypes.bfloat16
    pw = prim_w.reshape(2, 128, 2, 128, 9, 9)
    primw = (np.ascontiguousarray(
        pw.transpose(4, 5, 3, 2, 0, 1).reshape(81, 128, 512))
        * f32(2.0)).astype(bf16)
    primb2 = np.ascontiguousarray(prim_b.reshape(2, 128).T) * f32(2.0)

    # W2[i, r, co] with co = o*10 + c, zero-padded to K=32 route-quads:
    # w2g[g, rr*8+i, rr*160+co] = 2*W2[i, g*16+rr, co]
    Wt = np.ascontiguousarray(
        W.transpose(3, 0, 2, 1)).reshape(8, R, CO) * f32(2.0)

    # per-partition digit bias (2x domain), col0 = main co, col1 = tail fold
    bias_o = bias[:, 0]
    biav = np.zeros((128, 2), f32)
    for p in range(128):
        biav[p, 0] = f32(2.0) * bias_o[p // 10]
        biav[p, 1] = f32(2.0) * bias_o[(128 + p % 32) // 10]

    cos = np.arange(CO)
    sela = np.ascontiguousarray(
        (np.equal.outer(cos[:128] % 10, cos % 10)).astype(bf16))
    selt = np.ascontiguousarray(
        (np.equal.outer(cos[128:] % 10, cos % 10)).astype(bf16))
    # repl[k, m] = [k == m % 32]  (replicate [32,x] -> [128,x] via PE)
    repl = np.ascontiguousarray(
        np.equal.outer(np.arange(32), np.arange(128) % 32).astype(bf16))
    # fold[p, m] = [p % 32 == m]  (sum 4 rq-groups of partitions)
    fold = np.ascontiguousarray(
        np.equal.outer(np.arange(128) % 32, np.arange(32)).astype(f32))

    in_maps = []
    for k in range(N_CORES):
        im2 = np.ascontiguousarray(
            im2_all[BL * k:BL * (k + 1)].transpose(1, 0, 2).reshape(81, 1600))
        w2core = Wt[:, RL * k:RL * (k + 1), :]          # [8, 144, 160]
        w2g = np.zeros((9, 128, 16 * CO), bf16)
        for rr in range(16):
            # [8, 9, 160] block for this rr across all 9 groups
            blk = w2core[:, rr::16, :]
            w2g[:, rr * 8:(rr + 1) * 8, rr * CO:(rr + 1) * CO] = \
                blk.transpose(1, 0, 2)
        in_maps.append({
            "im2": im2, "convw": convw, "convb": convb2,
            "primw": primw, "primb": primb2, "w2g": w2g,
            "biav": biav, "sela": sela, "selt": selt,
            "repl": repl, "fold": fold,
        })
    return in_maps


HOST_SUM_OUT = True


def _postprocess(outm):
    """outm [160, 32] (co = o*10+c) -> classes [32, 10]."""
    out3 = outm.reshape(16, 10, 32).astype(np.float32) / np.float32(T)
    sq = (out3 * out3).sum(axis=0)
    return np.sqrt(sq).T.astype(np.float32)


def kernel(data, conv_w, conv_b, prim_w, prim_b, W, bias, time_window):
    from concourse.bass_utils import run_bass_kernel_spmd
    assert int(time_window) == T
    if "nc" not in _CACHE:
        _CACHE["nc"] = _build_program()
    nc = _CACHE["nc"]
    in_maps = _host_prepare(data, conv_w, conv_b, prim_w, prim_b, W, bias)
    res = run_bass_kernel_spmd(nc, in_maps, core_ids=list(range(N_CORES)))
    outm = np.sum([np.asarray(res.results[k]["outm"], np.float32)
                   for k in range(N_CORES)], axis=0)
    return _postprocess(outm)
